# revision 60
# baseline (speedup 1.0000x reference)
"""Tensor-parallel GQA attention kernel for 8 Trainium2 NeuronCores.

Sharding: head-parallel. Core c computes q heads [4c, 4c+4) and kv head c
(GQA group). The output projection is COLUMN-sharded: after each seq tile's
attention, a small AllGather (0.5 MB/core) exchanges the normalized ctx
features, and every core contracts all 4096 features against its wo
column-shard, writing final output columns [512c, 512c+512) directly. The
host reassembles the 4 x 8 (seq x column) blocks. This replaces the old
row-sharded partial-sum ReduceScatter chain (16 MB/core of collective
traffic that serialized ~90us past the last matmul).

Attention processes query heads in pairs; the QKV projection, scores, PV,
and denominator matmuls all use fp16 operands (LDWEIGHTS for 16-bit
stationaries is ~2x faster, giving full 213ns/matmul cadence), while PSUM
accumulation stays fp32 and the output projection keeps wo in f32r. The
softmax denominator is accumulated exactly on the PE via an all-ones fp16
stationary into a broadcast PSUM tile.
"""

import math
import sys

import numpy as np

sys.path.insert(0, "/opt/trn_rl_repo")

# ---- problem constants (hardcoded per harness contract) ----
DIM = 4096
N_HEADS = 32
N_KV_HEADS = 8
HEAD_DIM = 128
N_REP = 4
SEQ = 2048
BATCH = 1
NCORES = 8

P = 128
KO = DIM // P        # 32 contraction chunks
SQ = 512             # seq tile width (matmul moving free dim)
NSQ = SEQ // SQ      # 4
NKS = SEQ // P       # 16 key tiles of 128
NH_LOC = N_HEADS // NCORES   # 4 local q heads
MQKV = NH_LOC * HEAD_DIM + 2 * HEAD_DIM  # 768 rows of fused qkv projection
SCALE = 1.0 / math.sqrt(HEAD_DIM)
OSH = SQ // NCORES   # 64 seq rows per core from each ReduceScatter

XB = 4               # k-chunks per xT load (1 MB DMAs)
JORDER = (1, 2, 3, 0)  # j=0 last: its partials are the cheapest to compute
                       # at the tail, and processing j=1 first avoids a
                       # framework serialization of the first RS against the
                       # following tile's partial writes seen with j=0 first

_CACHE = {}


def _build():
    """Build and compile the Bass kernel once per process."""
    if "nc" in _CACHE:
        return _CACHE["nc"]

    import concourse.bacc as bacc
    import concourse.mybir as mybir
    import concourse.tile as tile
    from concourse.masks import make_identity
    from contextlib import ExitStack

    F32 = mybir.dt.float32
    F32R = mybir.dt.float32r
    F16 = mybir.dt.float16
    MULT = mybir.AluOpType.mult
    ADD = mybir.AluOpType.add
    SUB = mybir.AluOpType.subtract
    EXP = mybir.ActivationFunctionType.Exp

    nc = bacc.Bacc(None, target_bir_lowering=False, debug=False)

    xT = nc.declare_dram_parameter("xt", [P, NSQ, KO, SQ], F16, isOutput=False)
    wqkv = nc.declare_dram_parameter("wqkv", [P, KO, MQKV], F16, isOutput=False)
    # wo column-shard, feature-major: wo2[p, k, o] = wo[512c+o, 128k+p]
    wo2 = nc.declare_dram_parameter("wo2", [P, KO, SQ], F16, isOutput=False)
    cosd = nc.declare_dram_parameter("cost", [P, SEQ], F32, isOutput=False)
    sind = nc.declare_dram_parameter("sint", [P, SEQ], F32, isOutput=False)
    maskd = nc.declare_dram_parameter("masks", [P, 4, 2 * SQ], F16, isOutput=False)
    # each core emits the full seq rows of its 512 output COLUMNS: the out
    # projection is done locally against a wo column-shard after a small
    # AllGather of ctx (0.5 MB/core/tile), replacing the 16 MB/core serial
    # ReduceScatter chain that used to trail the compute by ~90us
    outs = [nc.declare_dram_parameter(f"o{j}", [SQ, SQ], F16, isOutput=True)
            for j in range(NSQ)]

    with tile.TileContext(nc) as tc, ExitStack() as stack:
        singles = stack.enter_context(tc.tile_pool(name="singles", bufs=1))
        dram = stack.enter_context(tc.tile_pool(name="dram", bufs=1, space="DRAM"))

        # AllGather staging, one collective per (seq tile, head pair): the
        # hp0 gather runs during hp1's attention, halving the AG latency
        # left exposed after each attention tile. agout rows are rank-
        # concatenated 2-head blocks; wo2's host layout matches that order.
        agins = [[dram.tile([2 * P, SQ], F16, name=f"agin{j}h{hp}")
                  for hp in range(2)] for j in range(NSQ)]
        agouts = [[dram.tile([2 * P * NCORES, SQ], F16, name=f"agout{j}h{hp}",
                             addr_space="Shared") for hp in range(2)]
                  for j in range(NSQ)]
        # warm-up collective, SAME shape as the real per-tile AllGathers: the
        # first collective after load pays an ~11us ncfw cold-start and the
        # first AG of a given size measured ~10us slower than later ones;
        # absorb both during phase 1 (result unused, input garbage is fine)
        warmin = dram.tile([2 * P, SQ], F16, name="warmin")
        warmout = dram.tile([2 * P * NCORES, SQ], F16, name="warmout",
                            addr_space="Shared")

        idn = singles.tile([P, P], F32)
        make_identity(nc, idn)

        warmsb = singles.tile([8, P], F16)
        nc.vector.memset(warmsb[:], 0.0)
        nc.sync.dma_start(warmin[0:8, 0:P], warmsb[:])
        nc.gpsimd.collective_compute(
            "AllGather", mybir.AluOpType.bypass,
            replica_groups=[list(range(NCORES))],
            ins=[warmin[:]], outs=[warmout[:]])

        ones_f = singles.tile([P, P], F32)
        nc.vector.memset(ones_f[:], 1.0)
        expbias = singles.tile([P, 1], F32)
        nc.vector.memset(expbias[:], -7.0)
        ones128 = singles.tile([P, P], F16)
        nc.vector.tensor_copy(ones128[:], ones_f[:])

        # attention operands, resident across phases 1-2
        qsb = singles.tile([P, NH_LOC, SEQ], F16)   # per head: rows 0:64 re, 64:128 im
        kTsb = singles.tile([P, SEQ], F16)
        vsb = singles.tile([P, NKS, HEAD_DIM], F16)
        # mask lives through phase 1 so its DMA can be issued early: the
        # first attention tile (j=0, t=0) is diagonal and needs it at once
        mask_sb = singles.tile([P, 4, 2 * SQ], F16)
        # the phase-2 softmax-path pools are created BEFORE phase 1 so their
        # tiles get dedicated addresses: created afterwards, they reuse the
        # freed phase-1 scratch and the first exp inherits a ~10us WAR wait
        # on the last sq tile's serial RoPE arithmetic chain
        ptpool = stack.enter_context(tc.tile_pool(name="pt", bufs=3))
        stpool = stack.enter_context(tc.tile_pool(name="st", bufs=2))
        cxpool = stack.enter_context(tc.tile_pool(name="cx", bufs=2))

        # ---------------- Phase 1: fused QKV projection + RoPE ----------------
        # m-tile order chosen so PSUM tiles are revisited in the order the
        # RoPE eviction frees them (pairs (0,3), (1,4), (2,5)).
        M_ORDER = (0, 3, 1, 4, 2, 5)
        with tc.tile_pool(name="wq", bufs=1) as wpool, \
             tc.tile_pool(name="xtp", bufs=2) as xpool, \
             tc.tile_pool(name="rt", bufs=2) as rpool, \
             tc.tile_pool(name="ps1", bufs=1, space="PSUM") as pp1:
            cos_sb = wpool.tile([P, SEQ], F32, tag="cos", name="cos_sb")
            sin_sb = wpool.tile([P, SEQ], F32, tag="sin", name="sin_sb")
            vTsb = wpool.tile([P, SEQ], F32, tag="vT", name="vTsb")

            # weight tiles allocated up front; DMAs interleaved with the x
            # stream of the first sq tile so the first matmul starts after
            # ~2.6 MB instead of 13 MB
            w = [wpool.tile([P, 4, MQKV], F16, tag=f"w{g}", name=f"w{g}")
                 for g in range(KO // 4)]
            # first weight/x chunks split in half so the very first matmuls
            # wait on ~0.9 MB of DMA instead of 1.8 MB
            nc.sync.dma_start(w[0][:, 0:2, :], wqkv[:, 0:2, :])

            def wslice(k, m):
                return w[k // 4][:, k % 4, m * P:(m + 1) * P]

            pending_tr = [None]
            for sq in range(NSQ):
                cols = slice(sq * SQ, (sq + 1) * SQ)
                # allocate in eviction-pair order so the banks freed first by
                # the RoPE chain are the ones phase 2's first tiles land on
                pq = [None] * 6
                for m in (0, 3, 1, 4, 2, 5):
                    pq[m] = pp1.tile([P, SQ], F32, tag=f"p{m}", name=f"p{m}_{sq}")
                for xb in range(KO // XB):
                    xk = xpool.tile([P, XB, SQ], F16, tag="xt", name=f"x{sq}_{xb}")
                    if sq == 0 and xb == 0:
                        nc.sync.dma_start(xk[:, 0:2, :], xT[:, 0, 0:2, :])
                        nc.sync.dma_start(w[0][:, 2:4, :], wqkv[:, 2:4, :])
                        nc.sync.dma_start(xk[:, 2:4, :], xT[:, 0, 2:4, :])
                    else:
                        nc.sync.dma_start(xk[:],
                                          xT[:, sq, xb * XB:(xb + 1) * XB, :])
                    if sq == 0 and xb + 1 < KO // 4:
                        nc.sync.dma_start(w[xb + 1][:],
                                          wqkv[:, 4 * (xb + 1):4 * (xb + 2), :])
                    # cos/sin aren't needed until the first RoPE eviction
                    # (~55us in) and the mask not until attention: spread
                    # them through the x/w stream so the DMA-paced first sq
                    # tile's chunks are never pushed back by >1 MB at once
                    if sq == 0 and xb == 2:
                        nc.sync.dma_start(cos_sb[:], cosd[:])
                    if sq == 0 and xb == 3:
                        nc.sync.dma_start(sin_sb[:], sind[:])
                    if sq == 0 and xb == 4:
                        nc.sync.dma_start(mask_sb[:], maskd[:])
                    for kk in range(XB):
                        k = xb * XB + kk
                        for m in M_ORDER:
                            nc.tensor.matmul(pq[m][:], wslice(k, m), xk[:, kk, :],
                                             start=(k == 0), stop=(k == KO - 1))
                    if xb == 0 and pending_tr[0] is not None:
                        pending_tr[0]()
                        pending_tr[0] = None

                # RoPE eviction. m-tile pairs: (0,3)->(q0,q1), (1,4)->(q2,q3),
                # (2,5)->(k | v-halves). Copy-first: ALL six PSUM->SBUF
                # copies are emitted before any arithmetic (vector takes the
                # A tiles, scalar the B tiles, in parallel), so every PSUM
                # bank is free within ~2us of the last matmul and the next
                # sq tile / phase 2 never stalls behind the RoPE math.
                pas, pbs = [], []
                for i in range(3):
                    pa = rpool.tile([P, SQ], F32, tag=f"pa{i}", name=f"pa{i}")
                    pb = rpool.tile([P, SQ], F32, tag=f"pb{i}", name=f"pb{i}")
                    nc.vector.tensor_copy(pa[:], pq[i][:])
                    nc.scalar.copy(pb[:], pq[i + 3][:])
                    pas.append(pa)
                    pbs.append(pb)
                # v passthrough immediately after the evictions: the v
                # transposes (PE queue) and everything behind them would
                # otherwise wait for the whole serial RoPE arithmetic chain
                nc.vector.tensor_copy(vTsb[0:64, cols], pas[2][64:128])
                nc.scalar.copy(vTsb[64:128, cols], pbs[2][64:128])
                for i, (h0, h1) in enumerate(((0, 1), (2, 3), (4, 5))):
                    pa, pb = pas[i], pbs[i]
                    tac = rpool.tile([P, SQ], F32, tag="tac")   # A*cos
                    tas = rpool.tile([P, SQ], F32, tag="tas")   # A*sin
                    tbs = rpool.tile([P, SQ], F32, tag="tbs")   # B*sin
                    tbc = rpool.tile([P, SQ], F32, tag="tbc")   # B*cos
                    nc.vector.tensor_tensor(tac[:], pa[:], cos_sb[:, cols],
                                            MULT)
                    nc.vector.tensor_tensor(tas[:], pa[:], sin_sb[:, cols],
                                            MULT)
                    nc.vector.tensor_tensor(tbs[:], pb[:], sin_sb[:, cols],
                                            MULT)
                    nc.vector.tensor_tensor(tbc[:], pb[:], cos_sb[:, cols],
                                            MULT)
                    if i == 2:
                        dests = ((slice(0, 64), kTsb[0:64, cols],
                                  kTsb[64:128, cols]),)
                    else:
                        h0q, h1q = 2 * i, 2 * i + 1
                        dests = ((slice(0, 64), qsb[0:64, h0q, cols],
                                  qsb[64:128, h0q, cols]),
                                 (slice(64, 128), qsb[0:64, h1q, cols],
                                  qsb[64:128, h1q, cols]))
                    for half, dre, dim_ in dests:
                        nc.vector.tensor_tensor(dre, tac[half], tbs[half], SUB)
                        nc.vector.tensor_tensor(dim_, tas[half], tbc[half], ADD)

                # transpose this quarter's v chunks: vT [128, s] -> v [s, 128]
                # (deferred into the next sq tile's matmul stream so the PE
                # doesn't stall here waiting for the RoPE vector ops)
                def mk_transposes(sq=sq):
                    def emit():
                        for t in range(4 * sq, 4 * sq + 4):
                            ptr = pp1.tile([P, P], F32, tag="ptr", bufs=2,
                                           name=f"ptr{t}")
                            nc.tensor.transpose(ptr[:],
                                                vTsb[:, t * P:(t + 1) * P],
                                                idn[:])
                            nc.scalar.copy(vsb[:, t, :], ptr[:])
                    return emit
                pending_tr[0] = mk_transposes()
                if sq == NSQ - 1:
                    pending_tr[0]()
                    pending_tr[0] = None

        wopool = stack.enter_context(tc.tile_pool(name="wopool", bufs=1))
        wo2_sb = wopool.tile([P, KO, SQ], F16)
        nc.sync.dma_start(wo2_sb[:], wo2[:])

        # ------- Phase 2+3: causal GQA attention + column-sharded out proj ----
        # emit_attn(j) ends by exporting normalized ctx and triggering its
        # AllGather; emit_p3l(j) stages the gathered 4096-feature ctx and
        # multiplies it against the wo column-shard, writing final output
        # rows directly (no ReduceScatter, no bounce copies)
        with tc.tile_pool(name="ag", bufs=2) as agpool, \
             tc.tile_pool(name="os", bufs=4) as ospool, \
             tc.tile_pool(name="ps2", bufs=1, space="PSUM") as pp2:

            def emit_attn(j):
                nks = 4 * (j + 1)
                ctx_sb = cxpool.tile([P, NH_LOC, SQ], F16, tag="cx",
                                     name=f"cx{j}")
                for hp in range(2):
                    h0, h1 = 2 * hp, 2 * hp + 1
                    ctx0 = pp2.tile([P, SQ], F32, tag="ctx", bufs=2,
                                    name=f"ctx{j}_{h0}")
                    ctx1 = pp2.tile([P, SQ], F32, tag="ctx", bufs=2,
                                    name=f"ctx{j}_{h1}")
                    # softmax denominator accumulated on the Vector engine
                    # (fp16 operands run at the 2x DVE rate); saves two PE
                    # matmuls per key tile
                    acc = stpool.tile([P, 2 * SQ], F16, tag="acc", bufs=2,
                                      name=f"acc{j}_{hp}")

                    # software pipeline: scores/exp run 2 tiles ahead of PV.
                    # Diagonal-block tiles (r = t-4j > 0) only attend queries
                    # q >= 128r, so scores/exp/mask/acc/PV are restricted to
                    # that column range (the excluded columns are exactly the
                    # fully-masked ones; acc/ctx keep their per-element
                    # accumulation correct because t=0 is always full-width)
                    def do_scores(t, j=j, h0=h0, h1=h1, acc=acc):
                        r = t - 4 * j if t >= 4 * j else 0
                        q0 = 128 * r
                        qc = slice(j * SQ + q0, (j + 1) * SQ)
                        ps_s = pp2.tile([P, 2 * SQ], F32, tag="s", bufs=2,
                                        name=f"s{j}_{h0}_{t}")
                        kt = kTsb[:, t * P:(t + 1) * P]
                        nc.tensor.matmul(ps_s[:, q0:SQ], kt, qsb[:, h0, qc],
                                         start=True, stop=True)
                        nc.tensor.matmul(ps_s[:, SQ + q0:], kt,
                                         qsb[:, h1, qc],
                                         start=True, stop=True)
                        pT = ptpool.tile([P, 2 * SQ], F16, tag="pT",
                                         name=f"pT{j}_{h0}_{t}")
                        # bias -7 keeps exp within fp16 range (max observed
                        # score*scale is ~11.5); numerator and denominator
                        # scale by the same e^-7, so softmax is unchanged
                        if q0 == 0:
                            nc.scalar.activation(pT[:], ps_s[:], EXP,
                                                 scale=SCALE, bias=expbias[:])
                        else:
                            nc.scalar.activation(pT[:, q0:SQ], ps_s[:, q0:SQ],
                                                 EXP, scale=SCALE,
                                                 bias=expbias[:])
                            nc.scalar.activation(pT[:, SQ + q0:],
                                                 ps_s[:, SQ + q0:], EXP,
                                                 scale=SCALE, bias=expbias[:])
                        if t >= 4 * j:
                            if q0 == 0:
                                nc.vector.tensor_tensor(
                                    pT[:], pT[:], mask_sb[:, r, :], MULT)
                            else:
                                nc.vector.tensor_tensor(
                                    pT[:, q0:SQ], pT[:, q0:SQ],
                                    mask_sb[:, r, q0:SQ], MULT)
                                nc.vector.tensor_tensor(
                                    pT[:, SQ + q0:], pT[:, SQ + q0:],
                                    mask_sb[:, r, SQ + q0:], MULT)
                        if t == 0:
                            nc.vector.tensor_copy(acc[:], pT[:])
                        elif q0 == 0:
                            nc.vector.tensor_tensor(acc[:], acc[:], pT[:],
                                                    ADD)
                        else:
                            nc.vector.tensor_tensor(acc[:, q0:SQ],
                                                    acc[:, q0:SQ],
                                                    pT[:, q0:SQ], ADD)
                            nc.vector.tensor_tensor(acc[:, SQ + q0:],
                                                    acc[:, SQ + q0:],
                                                    pT[:, SQ + q0:], ADD)
                        return pT

                    def do_pv(t, pT, ctx0=ctx0, ctx1=ctx1, nks=nks, j=j):
                        r = t - 4 * j if t >= 4 * j else 0
                        q0 = 128 * r
                        vt = vsb[:, t, :]
                        nc.tensor.matmul(ctx0[:, q0:], vt, pT[:, q0:SQ],
                                         start=(t == 0), stop=(t == nks - 1))
                        nc.tensor.matmul(ctx1[:, q0:], vt, pT[:, SQ + q0:],
                                         start=(t == 0), stop=(t == nks - 1))

                    pend = {}
                    for t in range(nks):
                        pend[t] = do_scores(t)
                        if t >= 2:
                            do_pv(t - 2, pend.pop(t - 2))
                    for t in (nks - 2, nks - 1):
                        do_pv(t, pend.pop(t))

                    # broadcast the denominator across partitions with an
                    # all-ones stationary, then normalize
                    bc = pp2.tile([P, 2 * SQ], F32, tag="bc", bufs=1,
                                  name=f"bc{j}_{hp}")
                    nc.tensor.matmul(bc[:, 0:SQ], ones128[:], acc[:, 0:SQ],
                                     start=True, stop=True)
                    nc.tensor.matmul(bc[:, SQ:], ones128[:], acc[:, SQ:],
                                     start=True, stop=True)
                    rc = stpool.tile([P, 2 * SQ], F32, tag="rc",
                                     name=f"rc{j}_{hp}")
                    nc.vector.reciprocal_approx_fast(rc[:], bc[:])
                    nc.vector.tensor_tensor(ctx_sb[:, h0, :], ctx0[:],
                                            rc[:, 0:SQ], MULT)
                    nc.vector.tensor_tensor(ctx_sb[:, h1, :], ctx1[:],
                                            rc[:, SQ:], MULT)
                    # export this head-pair's normalized ctx and gather it
                    # across cores immediately (0.25 MB in, 2 MB out)
                    nc.sync.dma_start(
                        agins[j][hp][:].rearrange("(h p) s -> p h s", h=2),
                        ctx_sb[:, h0:h1 + 1, :])
                    nc.gpsimd.collective_compute(
                        "AllGather", mybir.AluOpType.bypass,
                        replica_groups=[list(range(NCORES))],
                        ins=[agins[j][hp][:]], outs=[agouts[j][hp][:]])

            def emit_p3l(j):
                # local out projection for this seq tile: out[s, o_shard] =
                # sum_f ctx_all[f, s] * wo2[f, o]; fp32 PSUM over all 4096
                # features (numerically cleaner than the old fp16 partial
                # sum via the collective's CCE adds)
                agsb = agpool.tile([P, KO, SQ], F16, tag="agsb",
                                   name=f"agsb{j}")
                # staged in quarters so the first s-tile sweep pipelines
                # with the tail of the transfer; chunks 0:16 are the hp0
                # gather (2-head blocks per rank), 16:32 the hp1 gather
                for hp in range(2):
                    src = agouts[j][hp][:].rearrange("(f p) s -> p f s", p=P)
                    for b in range(2):
                        nc.sync.dma_start(
                            agsb[:, 16 * hp + 8 * b:16 * hp + 8 * (b + 1), :],
                            src[:, 8 * b:8 * (b + 1), :])
                for st in range(4):
                    # po2 shares the (double-buffered) s-tag banks: the next
                    # sweep's matmuls overlap this one's eviction
                    po2 = pp2.tile([P, 2 * SQ], F32, tag="s", bufs=2,
                                   name=f"po2_{j}_{st}")
                    for fc in range(KO):
                        nc.tensor.matmul(
                            po2[:, 0:SQ],
                            agsb[:, fc, st * P:(st + 1) * P],
                            wo2_sb[:, fc, :],
                            start=(fc == 0), stop=(fc == KO - 1))
                    osb = ospool.tile([P, SQ], F16, tag="osb",
                                      name=f"osb{j}_{st}")
                    nc.vector.tensor_copy(osb[:], po2[:, 0:SQ])
                    nc.sync.dma_start(outs[j][st * P:(st + 1) * P, :],
                                      osb[:])

            # interleave so each tile's AllGather + staging hides under the
            # next tile's attention; the final p3l pair runs back-to-back
            emit_attn(JORDER[0])
            emit_attn(JORDER[1])
            emit_p3l(JORDER[0])
            emit_attn(JORDER[2])
            emit_p3l(JORDER[1])
            emit_attn(JORDER[3])
            emit_p3l(JORDER[2])
            emit_p3l(JORDER[3])

    nc.compile()
    _CACHE["nc"] = nc
    return nc


def _prep_inputs(x, wq, wk, wv, wo, freqs_cos, freqs_sin):
    """Host-side sharding + layout prep. Returns in_maps for the 8 cores."""
    x = np.asarray(x, dtype=np.float32)
    wq = np.asarray(wq, dtype=np.float32)
    wk = np.asarray(wk, dtype=np.float32)
    wv = np.asarray(wv, dtype=np.float32)
    wo = np.asarray(wo, dtype=np.float32)
    freqs_cos = np.asarray(freqs_cos, dtype=np.float32)
    freqs_sin = np.asarray(freqs_sin, dtype=np.float32)

    # xT in [P, NSQ, KO, SQ] layout: element (d, s), d = ko*128 + p, s = sq*SQ + s'
    xT = np.ascontiguousarray(
        x[0].T.reshape(KO, P, NSQ, SQ).transpose(1, 2, 0, 3)).astype(np.float16)

    # rotate-half permutation within a head: [0,2,4,...126, 1,3,...,127]
    perm = np.concatenate([np.arange(0, HEAD_DIM, 2), np.arange(1, HEAD_DIM, 2)])

    # cos/sin tables transposed and duplicated across both 64-row halves
    cosT = np.ascontiguousarray(freqs_cos.T)  # [64, SEQ]
    sinT = np.ascontiguousarray(freqs_sin.T)
    cos2 = np.concatenate([cosT, cosT], axis=0)  # [128, SEQ]
    sin2 = np.concatenate([sinT, sinT], axis=0)

    # causal mask tiles: mask_r[i, jl] = 1 if jl - i >= 128*r, duplicated
    # across both halves of the head-pair score tile
    i_idx = np.arange(P)[:, None]
    j_idx = np.arange(SQ)[None, :]
    masks = np.stack([(j_idx - i_idx >= P * r).astype(np.float32)
                      for r in range(4)], axis=0)  # [4, 128, SQ]
    masks_l = np.ascontiguousarray(
        np.concatenate([masks, masks], axis=2).transpose(1, 0, 2)
    ).astype(np.float16)  # [P,4,2SQ]

    in_maps = []
    for c in range(NCORES):
        # fused qkv weight rows, permuted for RoPE (re/im separated by m-tile)
        qh = [wq[(4 * c + h) * HEAD_DIM:(4 * c + h + 1) * HEAD_DIM][perm]
              for h in range(NH_LOC)]  # each [128, DIM], rows [re(64); im(64)]
        kh = wk[c * HEAD_DIM:(c + 1) * HEAD_DIM][perm]  # [128, DIM]
        vh = wv[c * HEAD_DIM:(c + 1) * HEAD_DIM]        # [128, DIM] original order
        rows = np.empty((MQKV, DIM), dtype=np.float32)
        rows[0:64] = qh[0][0:64]        # tile0: q0 re | q1 re
        rows[64:128] = qh[1][0:64]
        rows[128:192] = qh[2][0:64]     # tile1: q2 re | q3 re
        rows[192:256] = qh[3][0:64]
        rows[256:320] = kh[0:64]        # tile2: k re | v dims 0:64
        rows[320:384] = vh[0:64]
        rows[384:448] = qh[0][64:128]   # tile3: q0 im | q1 im
        rows[448:512] = qh[1][64:128]
        rows[512:576] = qh[2][64:128]   # tile4: q2 im | q3 im
        rows[576:640] = qh[3][64:128]
        rows[640:704] = kh[64:128]      # tile5: k im | v dims 64:128
        rows[704:768] = vh[64:128]
        wqkvT = np.ascontiguousarray(
            rows.T.reshape(KO, P, MQKV).transpose(1, 0, 2)
        ).astype(np.float16)  # [P, KO, MQKV]

        # wo COLUMN shard, feature-major, rows permuted to the per-head-pair
        # AllGather order: chunks 0:16 = (rank g, heads 4g/4g+1), 16:32 =
        # (rank g, heads 4g+2/4g+3)
        order = ([4 * (i // 2) + (i % 2) for i in range(16)]
                 + [4 * (i // 2) + 2 + (i % 2) for i in range(16)])
        wo2T = np.ascontiguousarray(
            wo[c * SQ:(c + 1) * SQ, :].T
            .reshape(KO, P, SQ)[order].transpose(1, 0, 2)).astype(np.float16)

        in_maps.append({
            "xt": xT,
            "wqkv": wqkvT,
            "wo2": wo2T,
            "cost": cos2,
            "sint": sin2,
            "masks": masks_l,
        })
    return in_maps


def run(inputs, trace=False, tmpdir=None):
    """Compile (cached), run on 8 cores, return (output, BassKernelResults)."""
    from concourse.bass_utils import run_bass_kernel_spmd

    nc = _build()
    in_maps = _prep_inputs(**inputs)
    res = run_bass_kernel_spmd(nc, in_maps, list(range(NCORES)),
                               trace=trace, tmpdir=tmpdir)
    out = np.empty((BATCH, SEQ, DIM), dtype=np.float32)
    for c in range(NCORES):
        for j in range(NSQ):
            out[0, j * SQ:(j + 1) * SQ, c * SQ:(c + 1) * SQ] = np.asarray(
                res.results[c][f"o{j}"], dtype=np.float32)
    return out, res


def kernel(**inputs) -> np.ndarray:
    out, _ = run(inputs)
    return out



# revision 61
# speedup vs baseline: 1.1707x; 1.1707x over previous
"""Tensor-parallel GQA attention kernel for 8 Trainium2 NeuronCores.

Sharding: head-parallel. Core c computes q heads [4c, 4c+4) and kv head c
(GQA group). The output projection is COLUMN-sharded: after each seq tile's
attention, a small AllGather (0.5 MB/core) exchanges the normalized ctx
features, and every core contracts all 4096 features against its wo
column-shard, writing final output columns [512c, 512c+512) directly. The
host reassembles the 4 x 8 (seq x column) blocks. This replaces the old
row-sharded partial-sum ReduceScatter chain (16 MB/core of collective
traffic that serialized ~90us past the last matmul).

Attention processes query heads in pairs; the QKV projection, scores, PV,
and denominator matmuls all use fp16 operands (LDWEIGHTS for 16-bit
stationaries is ~2x faster, giving full 213ns/matmul cadence), while PSUM
accumulation stays fp32 and the output projection keeps wo in f32r. The
softmax denominator is accumulated exactly on the PE via an all-ones fp16
stationary into a broadcast PSUM tile.
"""

import math
import sys

import numpy as np

sys.path.insert(0, "/opt/trn_rl_repo")

# ---- problem constants (hardcoded per harness contract) ----
DIM = 4096
N_HEADS = 32
N_KV_HEADS = 8
HEAD_DIM = 128
N_REP = 4
SEQ = 2048
BATCH = 1
NCORES = 8

P = 128
KO = DIM // P        # 32 contraction chunks
SQ = 512             # seq tile width (matmul moving free dim)
NSQ = SEQ // SQ      # 4
NKS = SEQ // P       # 16 key tiles of 128
NH_LOC = N_HEADS // NCORES   # 4 local q heads
MQKV = NH_LOC * HEAD_DIM + 2 * HEAD_DIM  # 768 rows of fused qkv projection
SCALE = 1.0 / math.sqrt(HEAD_DIM)
OSH = SQ // NCORES   # 64 seq rows per core from each ReduceScatter

XB = 4               # k-chunks per xT load (1 MB DMAs)
JORDER = (1, 2, 3, 0)  # j=0 last: its partials are the cheapest to compute
                       # at the tail, and processing j=1 first avoids a
                       # framework serialization of the first RS against the
                       # following tile's partial writes seen with j=0 first

_CACHE = {}


def _build():
    """Build and compile the Bass kernel once per process."""
    if "nc" in _CACHE:
        return _CACHE["nc"]

    import concourse.bacc as bacc
    import concourse.mybir as mybir
    import concourse.tile as tile
    from concourse.masks import make_identity
    from contextlib import ExitStack

    F32 = mybir.dt.float32
    F32R = mybir.dt.float32r
    F16 = mybir.dt.float16
    MULT = mybir.AluOpType.mult
    ADD = mybir.AluOpType.add
    SUB = mybir.AluOpType.subtract
    EXP = mybir.ActivationFunctionType.Exp

    nc = bacc.Bacc(None, target_bir_lowering=False, debug=False)

    xT = nc.declare_dram_parameter("xt", [P, NSQ, KO, SQ], F16, isOutput=False)
    wqkv = nc.declare_dram_parameter("wqkv", [P, KO, MQKV], F16, isOutput=False)
    # wo column-shard, feature-major: wo2[p, k, o] = wo[512c+o, 128k+p]
    wo2 = nc.declare_dram_parameter("wo2", [P, KO, SQ], F16, isOutput=False)
    cosd = nc.declare_dram_parameter("cost", [P, SEQ], F32, isOutput=False)
    sind = nc.declare_dram_parameter("sint", [P, SEQ], F32, isOutput=False)
    maskd = nc.declare_dram_parameter("masks", [P, 4, 2 * SQ], F16, isOutput=False)
    # each core emits the full seq rows of its 512 output COLUMNS: the out
    # projection is done locally against a wo column-shard after a small
    # AllGather of ctx (0.5 MB/core/tile), replacing the 16 MB/core serial
    # ReduceScatter chain that used to trail the compute by ~90us
    outs = [nc.declare_dram_parameter(f"o{j}", [SQ, SQ], F16, isOutput=True)
            for j in range(NSQ)]

    with tile.TileContext(nc) as tc, ExitStack() as stack:
        singles = stack.enter_context(tc.tile_pool(name="singles", bufs=1))
        dram = stack.enter_context(tc.tile_pool(name="dram", bufs=1, space="DRAM"))

        # AllGather staging: agin = my 512 ctx features (feature-major) for
        # this seq tile; agout = all 4096 features, rank-concatenated
        agins = [dram.tile([NH_LOC * P, SQ], F16, name=f"agin{j}")
                 for j in range(NSQ)]
        agouts = [dram.tile([DIM, SQ], F16, name=f"agout{j}",
                            addr_space="Shared") for j in range(NSQ)]
        # warm-up collective, SAME shape as the real per-tile AllGathers: the
        # first collective after load pays an ~11us ncfw cold-start and the
        # first AG of a given size measured ~10us slower than later ones;
        # absorb both during phase 1 (result unused, input garbage is fine)
        warmin = dram.tile([NH_LOC * P, SQ], F16, name="warmin")
        warmout = dram.tile([DIM, SQ], F16, name="warmout",
                            addr_space="Shared")

        idn = singles.tile([P, P], F32)
        make_identity(nc, idn)

        warmsb = singles.tile([8, P], F16)
        nc.vector.memset(warmsb[:], 0.0)
        nc.sync.dma_start(warmin[0:8, 0:P], warmsb[:])
        nc.gpsimd.collective_compute(
            "AllGather", mybir.AluOpType.bypass,
            replica_groups=[list(range(NCORES))],
            ins=[warmin[:]], outs=[warmout[:]])

        ones_f = singles.tile([P, P], F32)
        nc.vector.memset(ones_f[:], 1.0)
        expbias = singles.tile([P, 1], F32)
        nc.vector.memset(expbias[:], -7.0)
        ones128 = singles.tile([P, P], F16)
        nc.vector.tensor_copy(ones128[:], ones_f[:])

        # attention operands, resident across phases 1-2
        qsb = singles.tile([P, NH_LOC, SEQ], F16)   # per head: rows 0:64 re, 64:128 im
        kTsb = singles.tile([P, SEQ], F16)
        vsb = singles.tile([P, NKS, HEAD_DIM], F16)
        # mask lives through phase 1 so its DMA can be issued early: the
        # first attention tile (j=0, t=0) is diagonal and needs it at once
        mask_sb = singles.tile([P, 4, 2 * SQ], F16)
        # the phase-2 softmax-path pools are created BEFORE phase 1 so their
        # tiles get dedicated addresses: created afterwards, they reuse the
        # freed phase-1 scratch and the first exp inherits a ~10us WAR wait
        # on the last sq tile's serial RoPE arithmetic chain
        ptpool = stack.enter_context(tc.tile_pool(name="pt", bufs=3))
        stpool = stack.enter_context(tc.tile_pool(name="st", bufs=2))
        cxpool = stack.enter_context(tc.tile_pool(name="cx", bufs=2))

        # ---------------- Phase 1: fused QKV projection + RoPE ----------------
        # m-tile order chosen so PSUM tiles are revisited in the order the
        # RoPE eviction frees them (pairs (0,3), (1,4), (2,5)).
        M_ORDER = (0, 3, 1, 4, 2, 5)
        with tc.tile_pool(name="wq", bufs=1) as wpool, \
             tc.tile_pool(name="xtp", bufs=2) as xpool, \
             tc.tile_pool(name="rt", bufs=2) as rpool, \
             tc.tile_pool(name="ps1", bufs=1, space="PSUM") as pp1:
            cos_sb = wpool.tile([P, SEQ], F32, tag="cos", name="cos_sb")
            sin_sb = wpool.tile([P, SEQ], F32, tag="sin", name="sin_sb")
            vTsb = wpool.tile([P, SEQ], F32, tag="vT", name="vTsb")

            # weight tiles allocated up front; DMAs interleaved with the x
            # stream of the first sq tile so the first matmul starts after
            # ~2.6 MB instead of 13 MB
            w = [wpool.tile([P, 4, MQKV], F16, tag=f"w{g}", name=f"w{g}")
                 for g in range(KO // 4)]
            # first weight/x chunks split in half so the very first matmuls
            # wait on ~0.9 MB of DMA instead of 1.8 MB
            nc.sync.dma_start(w[0][:, 0:2, :], wqkv[:, 0:2, :])

            def wslice(k, m):
                return w[k // 4][:, k % 4, m * P:(m + 1) * P]

            pending_tr = [None]
            for sq in range(NSQ):
                cols = slice(sq * SQ, (sq + 1) * SQ)
                # allocate in eviction-pair order so the banks freed first by
                # the RoPE chain are the ones phase 2's first tiles land on
                pq = [None] * 6
                for m in (0, 3, 1, 4, 2, 5):
                    pq[m] = pp1.tile([P, SQ], F32, tag=f"p{m}", name=f"p{m}_{sq}")
                for xb in range(KO // XB):
                    xk = xpool.tile([P, XB, SQ], F16, tag="xt", name=f"x{sq}_{xb}")
                    if sq == 0 and xb == 0:
                        nc.sync.dma_start(xk[:, 0:2, :], xT[:, 0, 0:2, :])
                        nc.sync.dma_start(w[0][:, 2:4, :], wqkv[:, 2:4, :])
                        nc.sync.dma_start(xk[:, 2:4, :], xT[:, 0, 2:4, :])
                    else:
                        nc.sync.dma_start(xk[:],
                                          xT[:, sq, xb * XB:(xb + 1) * XB, :])
                    if sq == 0 and xb + 1 < KO // 4:
                        nc.sync.dma_start(w[xb + 1][:],
                                          wqkv[:, 4 * (xb + 1):4 * (xb + 2), :])
                    # cos/sin aren't needed until the first RoPE eviction
                    # (~55us in) and the mask not until attention: spread
                    # them through the x/w stream so the DMA-paced first sq
                    # tile's chunks are never pushed back by >1 MB at once
                    if sq == 0 and xb == 2:
                        nc.sync.dma_start(cos_sb[:], cosd[:])
                    if sq == 0 and xb == 3:
                        nc.sync.dma_start(sin_sb[:], sind[:])
                    if sq == 0 and xb == 4:
                        nc.sync.dma_start(mask_sb[:], maskd[:])
                    for kk in range(XB):
                        k = xb * XB + kk
                        for m in M_ORDER:
                            nc.tensor.matmul(pq[m][:], wslice(k, m), xk[:, kk, :],
                                             start=(k == 0), stop=(k == KO - 1))
                    if xb == 0 and pending_tr[0] is not None:
                        pending_tr[0]()
                        pending_tr[0] = None

                # RoPE eviction. m-tile pairs: (0,3)->(q0,q1), (1,4)->(q2,q3),
                # (2,5)->(k | v-halves). Copy-first: ALL six PSUM->SBUF
                # copies are emitted before any arithmetic (vector takes the
                # A tiles, scalar the B tiles, in parallel), so every PSUM
                # bank is free within ~2us of the last matmul and the next
                # sq tile / phase 2 never stalls behind the RoPE math.
                pas, pbs = [], []
                for i in range(3):
                    pa = rpool.tile([P, SQ], F32, tag=f"pa{i}", name=f"pa{i}")
                    pb = rpool.tile([P, SQ], F32, tag=f"pb{i}", name=f"pb{i}")
                    nc.vector.tensor_copy(pa[:], pq[i][:])
                    nc.scalar.copy(pb[:], pq[i + 3][:])
                    pas.append(pa)
                    pbs.append(pb)
                # v passthrough immediately after the evictions: the v
                # transposes (PE queue) and everything behind them would
                # otherwise wait for the whole serial RoPE arithmetic chain
                nc.vector.tensor_copy(vTsb[0:64, cols], pas[2][64:128])
                nc.scalar.copy(vTsb[64:128, cols], pbs[2][64:128])
                for i, (h0, h1) in enumerate(((0, 1), (2, 3), (4, 5))):
                    pa, pb = pas[i], pbs[i]
                    tac = rpool.tile([P, SQ], F32, tag="tac")   # A*cos
                    tas = rpool.tile([P, SQ], F32, tag="tas")   # A*sin
                    tbs = rpool.tile([P, SQ], F32, tag="tbs")   # B*sin
                    tbc = rpool.tile([P, SQ], F32, tag="tbc")   # B*cos
                    nc.vector.tensor_tensor(tac[:], pa[:], cos_sb[:, cols],
                                            MULT)
                    nc.vector.tensor_tensor(tas[:], pa[:], sin_sb[:, cols],
                                            MULT)
                    nc.vector.tensor_tensor(tbs[:], pb[:], sin_sb[:, cols],
                                            MULT)
                    nc.vector.tensor_tensor(tbc[:], pb[:], cos_sb[:, cols],
                                            MULT)
                    if i == 2:
                        dests = ((slice(0, 64), kTsb[0:64, cols],
                                  kTsb[64:128, cols]),)
                    else:
                        h0q, h1q = 2 * i, 2 * i + 1
                        dests = ((slice(0, 64), qsb[0:64, h0q, cols],
                                  qsb[64:128, h0q, cols]),
                                 (slice(64, 128), qsb[0:64, h1q, cols],
                                  qsb[64:128, h1q, cols]))
                    for half, dre, dim_ in dests:
                        nc.vector.tensor_tensor(dre, tac[half], tbs[half], SUB)
                        nc.vector.tensor_tensor(dim_, tas[half], tbc[half], ADD)

                # transpose this quarter's v chunks: vT [128, s] -> v [s, 128]
                # (deferred into the next sq tile's matmul stream so the PE
                # doesn't stall here waiting for the RoPE vector ops)
                def mk_transposes(sq=sq):
                    def emit():
                        for t in range(4 * sq, 4 * sq + 4):
                            ptr = pp1.tile([P, P], F32, tag="ptr", bufs=2,
                                           name=f"ptr{t}")
                            nc.tensor.transpose(ptr[:],
                                                vTsb[:, t * P:(t + 1) * P],
                                                idn[:])
                            nc.scalar.copy(vsb[:, t, :], ptr[:])
                    return emit
                pending_tr[0] = mk_transposes()
                if sq == NSQ - 1:
                    pending_tr[0]()
                    pending_tr[0] = None

        wopool = stack.enter_context(tc.tile_pool(name="wopool", bufs=1))
        wo2_sb = wopool.tile([P, KO, SQ], F16)
        nc.sync.dma_start(wo2_sb[:], wo2[:])

        # ------- Phase 2+3: causal GQA attention + column-sharded out proj ----
        # emit_attn(j) ends by exporting normalized ctx and triggering its
        # AllGather; emit_p3l(j) stages the gathered 4096-feature ctx and
        # multiplies it against the wo column-shard, writing final output
        # rows directly (no ReduceScatter, no bounce copies)
        with tc.tile_pool(name="ag", bufs=2) as agpool, \
             tc.tile_pool(name="os", bufs=4) as ospool, \
             tc.tile_pool(name="ps2", bufs=1, space="PSUM") as pp2:

            def emit_attn(j):
                nks = 4 * (j + 1)
                ctx_sb = cxpool.tile([P, NH_LOC, SQ], F16, tag="cx",
                                     name=f"cx{j}")
                for hp in range(2):
                    h0, h1 = 2 * hp, 2 * hp + 1
                    ctx0 = pp2.tile([P, SQ], F32, tag="ctx", bufs=2,
                                    name=f"ctx{j}_{h0}")
                    ctx1 = pp2.tile([P, SQ], F32, tag="ctx", bufs=2,
                                    name=f"ctx{j}_{h1}")
                    # softmax denominator accumulated on the Vector engine
                    # (fp16 operands run at the 2x DVE rate); saves two PE
                    # matmuls per key tile
                    acc = stpool.tile([P, 2 * SQ], F16, tag="acc", bufs=2,
                                      name=f"acc{j}_{hp}")

                    # software pipeline: scores/exp run 2 tiles ahead of PV.
                    # Diagonal-block tiles (r = t-4j > 0) only attend queries
                    # q >= 128r, so scores/exp/mask/acc/PV are restricted to
                    # that column range (the excluded columns are exactly the
                    # fully-masked ones; acc/ctx keep their per-element
                    # accumulation correct because t=0 is always full-width)
                    def do_scores(t, j=j, h0=h0, h1=h1, acc=acc):
                        r = t - 4 * j if t >= 4 * j else 0
                        q0 = 128 * r
                        qc = slice(j * SQ + q0, (j + 1) * SQ)
                        ps_s = pp2.tile([P, 2 * SQ], F32, tag="s", bufs=2,
                                        name=f"s{j}_{h0}_{t}")
                        kt = kTsb[:, t * P:(t + 1) * P]
                        nc.tensor.matmul(ps_s[:, q0:SQ], kt, qsb[:, h0, qc],
                                         start=True, stop=True)
                        nc.tensor.matmul(ps_s[:, SQ + q0:], kt,
                                         qsb[:, h1, qc],
                                         start=True, stop=True)
                        pT = ptpool.tile([P, 2 * SQ], F16, tag="pT",
                                         name=f"pT{j}_{h0}_{t}")
                        # bias -7 keeps exp within fp16 range (max observed
                        # score*scale is ~11.5); numerator and denominator
                        # scale by the same e^-7, so softmax is unchanged
                        if q0 == 0:
                            nc.scalar.activation(pT[:], ps_s[:], EXP,
                                                 scale=SCALE, bias=expbias[:])
                        else:
                            nc.scalar.activation(pT[:, q0:SQ], ps_s[:, q0:SQ],
                                                 EXP, scale=SCALE,
                                                 bias=expbias[:])
                            nc.scalar.activation(pT[:, SQ + q0:],
                                                 ps_s[:, SQ + q0:], EXP,
                                                 scale=SCALE, bias=expbias[:])
                        if t >= 4 * j:
                            if q0 == 0:
                                nc.vector.tensor_tensor(
                                    pT[:], pT[:], mask_sb[:, r, :], MULT)
                            else:
                                nc.vector.tensor_tensor(
                                    pT[:, q0:SQ], pT[:, q0:SQ],
                                    mask_sb[:, r, q0:SQ], MULT)
                                nc.vector.tensor_tensor(
                                    pT[:, SQ + q0:], pT[:, SQ + q0:],
                                    mask_sb[:, r, SQ + q0:], MULT)
                        if t == 0:
                            nc.vector.tensor_copy(acc[:], pT[:])
                        elif q0 == 0:
                            nc.vector.tensor_tensor(acc[:], acc[:], pT[:],
                                                    ADD)
                        else:
                            nc.vector.tensor_tensor(acc[:, q0:SQ],
                                                    acc[:, q0:SQ],
                                                    pT[:, q0:SQ], ADD)
                            nc.vector.tensor_tensor(acc[:, SQ + q0:],
                                                    acc[:, SQ + q0:],
                                                    pT[:, SQ + q0:], ADD)
                        return pT

                    def do_pv(t, pT, ctx0=ctx0, ctx1=ctx1, nks=nks, j=j):
                        r = t - 4 * j if t >= 4 * j else 0
                        q0 = 128 * r
                        vt = vsb[:, t, :]
                        nc.tensor.matmul(ctx0[:, q0:], vt, pT[:, q0:SQ],
                                         start=(t == 0), stop=(t == nks - 1))
                        nc.tensor.matmul(ctx1[:, q0:], vt, pT[:, SQ + q0:],
                                         start=(t == 0), stop=(t == nks - 1))

                    pend = {}
                    for t in range(nks):
                        pend[t] = do_scores(t)
                        if t >= 2:
                            do_pv(t - 2, pend.pop(t - 2))
                    for t in (nks - 2, nks - 1):
                        do_pv(t, pend.pop(t))

                    # broadcast the denominator across partitions with an
                    # all-ones stationary, then normalize
                    bc = pp2.tile([P, 2 * SQ], F32, tag="bc", bufs=1,
                                  name=f"bc{j}_{hp}")
                    nc.tensor.matmul(bc[:, 0:SQ], ones128[:], acc[:, 0:SQ],
                                     start=True, stop=True)
                    nc.tensor.matmul(bc[:, SQ:], ones128[:], acc[:, SQ:],
                                     start=True, stop=True)
                    rc = stpool.tile([P, 2 * SQ], F32, tag="rc",
                                     name=f"rc{j}_{hp}")
                    nc.vector.reciprocal_approx_fast(rc[:], bc[:])
                    nc.vector.tensor_tensor(ctx_sb[:, h0, :], ctx0[:],
                                            rc[:, 0:SQ], MULT)
                    nc.vector.tensor_tensor(ctx_sb[:, h1, :], ctx1[:],
                                            rc[:, SQ:], MULT)
                    # export this head-pair's normalized ctx immediately so
                    # the AllGather can trigger right at attention end
                    nc.sync.dma_start(
                        agins[j][:].rearrange("(h p) s -> p h s",
                                              h=NH_LOC)[:, h0:h1 + 1, :],
                        ctx_sb[:, h0:h1 + 1, :])

                # gather all cores' ctx features for this seq tile
                # (0.5 MB in, 4 MB out per core)
                nc.gpsimd.collective_compute(
                    "AllGather", mybir.AluOpType.bypass,
                    replica_groups=[list(range(NCORES))],
                    ins=[agins[j][:]], outs=[agouts[j][:]])

            def emit_p3l(j):
                # local out projection for this seq tile: out[s, o_shard] =
                # sum_f ctx_all[f, s] * wo2[f, o]; fp32 PSUM over all 4096
                # features (numerically cleaner than the old fp16 partial
                # sum via the collective's CCE adds)
                agsb = agpool.tile([P, KO, SQ], F16, tag="agsb",
                                   name=f"agsb{j}")
                src = agouts[j][:].rearrange("(f p) s -> p f s", p=P)
                # staged in quarters so the first s-tile sweep pipelines
                # with the tail of the transfer
                for b in range(4):
                    nc.sync.dma_start(agsb[:, 8 * b:8 * (b + 1), :],
                                      src[:, 8 * b:8 * (b + 1), :])
                for st in range(4):
                    # po2 shares the (double-buffered) s-tag banks: the next
                    # sweep's matmuls overlap this one's eviction
                    po2 = pp2.tile([P, 2 * SQ], F32, tag="s", bufs=2,
                                   name=f"po2_{j}_{st}")
                    for fc in range(KO):
                        nc.tensor.matmul(
                            po2[:, 0:SQ],
                            agsb[:, fc, st * P:(st + 1) * P],
                            wo2_sb[:, fc, :],
                            start=(fc == 0), stop=(fc == KO - 1))
                    osb = ospool.tile([P, SQ], F16, tag="osb",
                                      name=f"osb{j}_{st}")
                    nc.vector.tensor_copy(osb[:], po2[:, 0:SQ])
                    nc.sync.dma_start(outs[j][st * P:(st + 1) * P, :],
                                      osb[:])

            # interleave so each tile's AllGather + staging hides under the
            # next tile's attention; the final p3l pair runs back-to-back
            emit_attn(JORDER[0])
            emit_attn(JORDER[1])
            emit_p3l(JORDER[0])
            emit_attn(JORDER[2])
            emit_p3l(JORDER[1])
            emit_attn(JORDER[3])
            emit_p3l(JORDER[2])
            emit_p3l(JORDER[3])

    nc.compile()
    _CACHE["nc"] = nc
    return nc


def _prep_inputs(x, wq, wk, wv, wo, freqs_cos, freqs_sin):
    """Host-side sharding + layout prep. Returns in_maps for the 8 cores."""
    x = np.asarray(x, dtype=np.float32)
    wq = np.asarray(wq, dtype=np.float32)
    wk = np.asarray(wk, dtype=np.float32)
    wv = np.asarray(wv, dtype=np.float32)
    wo = np.asarray(wo, dtype=np.float32)
    freqs_cos = np.asarray(freqs_cos, dtype=np.float32)
    freqs_sin = np.asarray(freqs_sin, dtype=np.float32)

    # xT in [P, NSQ, KO, SQ] layout: element (d, s), d = ko*128 + p, s = sq*SQ + s'
    xT = np.ascontiguousarray(
        x[0].T.reshape(KO, P, NSQ, SQ).transpose(1, 2, 0, 3)).astype(np.float16)

    # rotate-half permutation within a head: [0,2,4,...126, 1,3,...,127]
    perm = np.concatenate([np.arange(0, HEAD_DIM, 2), np.arange(1, HEAD_DIM, 2)])

    # cos/sin tables transposed and duplicated across both 64-row halves
    cosT = np.ascontiguousarray(freqs_cos.T)  # [64, SEQ]
    sinT = np.ascontiguousarray(freqs_sin.T)
    cos2 = np.concatenate([cosT, cosT], axis=0)  # [128, SEQ]
    sin2 = np.concatenate([sinT, sinT], axis=0)

    # causal mask tiles: mask_r[i, jl] = 1 if jl - i >= 128*r, duplicated
    # across both halves of the head-pair score tile
    i_idx = np.arange(P)[:, None]
    j_idx = np.arange(SQ)[None, :]
    masks = np.stack([(j_idx - i_idx >= P * r).astype(np.float32)
                      for r in range(4)], axis=0)  # [4, 128, SQ]
    masks_l = np.ascontiguousarray(
        np.concatenate([masks, masks], axis=2).transpose(1, 0, 2)
    ).astype(np.float16)  # [P,4,2SQ]

    in_maps = []
    for c in range(NCORES):
        # fused qkv weight rows, permuted for RoPE (re/im separated by m-tile)
        qh = [wq[(4 * c + h) * HEAD_DIM:(4 * c + h + 1) * HEAD_DIM][perm]
              for h in range(NH_LOC)]  # each [128, DIM], rows [re(64); im(64)]
        kh = wk[c * HEAD_DIM:(c + 1) * HEAD_DIM][perm]  # [128, DIM]
        vh = wv[c * HEAD_DIM:(c + 1) * HEAD_DIM]        # [128, DIM] original order
        rows = np.empty((MQKV, DIM), dtype=np.float32)
        rows[0:64] = qh[0][0:64]        # tile0: q0 re | q1 re
        rows[64:128] = qh[1][0:64]
        rows[128:192] = qh[2][0:64]     # tile1: q2 re | q3 re
        rows[192:256] = qh[3][0:64]
        rows[256:320] = kh[0:64]        # tile2: k re | v dims 0:64
        rows[320:384] = vh[0:64]
        rows[384:448] = qh[0][64:128]   # tile3: q0 im | q1 im
        rows[448:512] = qh[1][64:128]
        rows[512:576] = qh[2][64:128]   # tile4: q2 im | q3 im
        rows[576:640] = qh[3][64:128]
        rows[640:704] = kh[64:128]      # tile5: k im | v dims 64:128
        rows[704:768] = vh[64:128]
        wqkvT = np.ascontiguousarray(
            rows.T.reshape(KO, P, MQKV).transpose(1, 0, 2)
        ).astype(np.float16)  # [P, KO, MQKV]

        # wo COLUMN shard, feature-major: wo2[p, k, o] = wo[512c+o, 128k+p]
        wo2T = np.ascontiguousarray(
            wo[c * SQ:(c + 1) * SQ, :].T
            .reshape(KO, P, SQ).transpose(1, 0, 2)).astype(np.float16)

        in_maps.append({
            "xt": xT,
            "wqkv": wqkvT,
            "wo2": wo2T,
            "cost": cos2,
            "sint": sin2,
            "masks": masks_l,
        })
    return in_maps


def run(inputs, trace=False, tmpdir=None):
    """Compile (cached), run on 8 cores, return (output, BassKernelResults)."""
    from concourse.bass_utils import run_bass_kernel_spmd

    nc = _build()
    in_maps = _prep_inputs(**inputs)
    res = run_bass_kernel_spmd(nc, in_maps, list(range(NCORES)),
                               trace=trace, tmpdir=tmpdir)
    out = np.empty((BATCH, SEQ, DIM), dtype=np.float32)
    for c in range(NCORES):
        for j in range(NSQ):
            out[0, j * SQ:(j + 1) * SQ, c * SQ:(c + 1) * SQ] = np.asarray(
                res.results[c][f"o{j}"], dtype=np.float32)
    return out, res


def kernel(**inputs) -> np.ndarray:
    out, _ = run(inputs)
    return out



# revision 64
# speedup vs baseline: 1.1749x; 1.0036x over previous
"""Tensor-parallel GQA attention kernel for 8 Trainium2 NeuronCores.

Sharding: head-parallel. Core c computes q heads [4c, 4c+4) and kv head c
(GQA group). The output projection is COLUMN-sharded: after each seq tile's
attention, a small AllGather (0.5 MB/core) exchanges the normalized ctx
features, and every core contracts all 4096 features against its wo
column-shard, writing final output columns [512c, 512c+512) directly. The
host reassembles the 4 x 8 (seq x column) blocks. This replaces the old
row-sharded partial-sum ReduceScatter chain (16 MB/core of collective
traffic that serialized ~90us past the last matmul).

Attention processes query heads in pairs; the QKV projection, scores, PV,
and denominator matmuls all use fp16 operands (LDWEIGHTS for 16-bit
stationaries is ~2x faster, giving full 213ns/matmul cadence), while PSUM
accumulation stays fp32 and the output projection keeps wo in f32r. The
softmax denominator is accumulated exactly on the PE via an all-ones fp16
stationary into a broadcast PSUM tile.
"""

import math
import sys

import numpy as np

sys.path.insert(0, "/opt/trn_rl_repo")

# ---- problem constants (hardcoded per harness contract) ----
DIM = 4096
N_HEADS = 32
N_KV_HEADS = 8
HEAD_DIM = 128
N_REP = 4
SEQ = 2048
BATCH = 1
NCORES = 8

P = 128
KO = DIM // P        # 32 contraction chunks
SQ = 512             # seq tile width (matmul moving free dim)
NSQ = SEQ // SQ      # 4
NKS = SEQ // P       # 16 key tiles of 128
NH_LOC = N_HEADS // NCORES   # 4 local q heads
MQKV = NH_LOC * HEAD_DIM + 2 * HEAD_DIM  # 768 rows of fused qkv projection
SCALE = 1.0 / math.sqrt(HEAD_DIM)
OSH = SQ // NCORES   # 64 seq rows per core from each ReduceScatter

XB = 4               # k-chunks per xT load (1 MB DMAs)
JORDER = (1, 2, 3, 0)  # j=0 last: its partials are the cheapest to compute
                       # at the tail, and processing j=1 first avoids a
                       # framework serialization of the first RS against the
                       # following tile's partial writes seen with j=0 first

_CACHE = {}


def _build():
    """Build and compile the Bass kernel once per process."""
    if "nc" in _CACHE:
        return _CACHE["nc"]

    import concourse.bacc as bacc
    import concourse.mybir as mybir
    import concourse.tile as tile
    from concourse.masks import make_identity
    from contextlib import ExitStack

    F32 = mybir.dt.float32
    F32R = mybir.dt.float32r
    F16 = mybir.dt.float16
    MULT = mybir.AluOpType.mult
    ADD = mybir.AluOpType.add
    SUB = mybir.AluOpType.subtract
    EXP = mybir.ActivationFunctionType.Exp

    nc = bacc.Bacc(None, target_bir_lowering=False, debug=False)

    xT = nc.declare_dram_parameter("xt", [P, NSQ, KO, SQ], F16, isOutput=False)
    wqkv = nc.declare_dram_parameter("wqkv", [P, KO, MQKV], F16, isOutput=False)
    # wo column-shard, feature-major: wo2[p, k, o] = wo[512c+o, 128k+p]
    wo2 = nc.declare_dram_parameter("wo2", [P, KO, SQ], F16, isOutput=False)
    cosd = nc.declare_dram_parameter("cost", [P, SEQ], F32, isOutput=False)
    sind = nc.declare_dram_parameter("sint", [P, SEQ], F32, isOutput=False)
    maskd = nc.declare_dram_parameter("masks", [P, 4, 2 * SQ], F16, isOutput=False)
    # each core emits the full seq rows of its 512 output COLUMNS: the out
    # projection is done locally against a wo column-shard after a small
    # AllGather of ctx (0.5 MB/core/tile), replacing the 16 MB/core serial
    # ReduceScatter chain that used to trail the compute by ~90us
    outs = [nc.declare_dram_parameter(f"o{j}", [SQ, SQ], F16, isOutput=True)
            for j in range(NSQ)]

    with tile.TileContext(nc) as tc, ExitStack() as stack:
        singles = stack.enter_context(tc.tile_pool(name="singles", bufs=1))
        dram = stack.enter_context(tc.tile_pool(name="dram", bufs=1, space="DRAM"))

        # AllGather staging: agin = my 512 ctx features (feature-major) for
        # this seq tile; agout = all 4096 features, rank-concatenated
        agins = [dram.tile([NH_LOC * P, SQ], F16, name=f"agin{j}")
                 for j in range(NSQ)]
        agouts = [dram.tile([DIM, SQ], F16, name=f"agout{j}",
                            addr_space="Shared") for j in range(NSQ)]
        # warm-up collective, SAME shape as the real per-tile AllGathers: the
        # first collective after load pays an ~11us ncfw cold-start and the
        # first AG of a given size measured ~10us slower than later ones;
        # absorb both during phase 1 (result unused, input garbage is fine)
        warmin = dram.tile([NH_LOC * P, SQ], F16, name="warmin")
        warmout = dram.tile([DIM, SQ], F16, name="warmout",
                            addr_space="Shared")

        idn = singles.tile([P, P], F32)
        make_identity(nc, idn)

        # attention operands, resident across phases 1-2
        qsb = singles.tile([P, NH_LOC, SEQ], F16)   # per head: rows 0:64 re, 64:128 im
        kTsb = singles.tile([P, SEQ], F16)
        vsb = singles.tile([P, NKS, HEAD_DIM], F16)
        mask_sb = singles.tile([P, 4, 2 * SQ], F16)
        cos_sb = singles.tile([P, SEQ], F32)
        sin_sb = singles.tile([P, SEQ], F32)

        # the cos/sin/mask tables load on the otherwise-idle gpsimd queue,
        # in parallel with the sync queue's weight/activation stream: the
        # first sq tile of phase 1 is DMA-paced, and pushing 3 MB of tables
        # off that queue removes ~12us of matmul-stream slippage
        nc.gpsimd.dma_start(cos_sb[:], cosd[:])
        nc.gpsimd.dma_start(sin_sb[:], sind[:])
        nc.gpsimd.dma_start(mask_sb[:], maskd[:])

        warmsb = singles.tile([8, P], F16)
        nc.vector.memset(warmsb[:], 0.0)
        nc.sync.dma_start(warmin[0:8, 0:P], warmsb[:])
        nc.gpsimd.collective_compute(
            "AllGather", mybir.AluOpType.bypass,
            replica_groups=[list(range(NCORES))],
            ins=[warmin[:]], outs=[warmout[:]])

        ones_f = singles.tile([P, P], F32)
        nc.vector.memset(ones_f[:], 1.0)
        expbias = singles.tile([P, 1], F32)
        nc.vector.memset(expbias[:], -7.0)
        ones128 = singles.tile([P, P], F16)
        nc.vector.tensor_copy(ones128[:], ones_f[:])
        # the phase-2 softmax-path pools are created BEFORE phase 1 so their
        # tiles get dedicated addresses: created afterwards, they reuse the
        # freed phase-1 scratch and the first exp inherits a ~10us WAR wait
        # on the last sq tile's serial RoPE arithmetic chain
        ptpool = stack.enter_context(tc.tile_pool(name="pt", bufs=3))
        stpool = stack.enter_context(tc.tile_pool(name="st", bufs=2))
        cxpool = stack.enter_context(tc.tile_pool(name="cx", bufs=2))

        # ---------------- Phase 1: fused QKV projection + RoPE ----------------
        # m-tile order chosen so PSUM tiles are revisited in the order the
        # RoPE eviction frees them (pairs (0,3), (1,4), (2,5)).
        M_ORDER = (0, 3, 1, 4, 2, 5)
        with tc.tile_pool(name="wq", bufs=1) as wpool, \
             tc.tile_pool(name="xtp", bufs=2) as xpool, \
             tc.tile_pool(name="rt", bufs=2) as rpool, \
             tc.tile_pool(name="ps1", bufs=1, space="PSUM") as pp1:
            vTsb = wpool.tile([P, SEQ], F32, tag="vT", name="vTsb")

            # weight tiles allocated up front; DMAs interleaved with the x
            # stream of the first sq tile so the first matmul starts after
            # ~2.6 MB instead of 13 MB
            w = [wpool.tile([P, 4, MQKV], F16, tag=f"w{g}", name=f"w{g}")
                 for g in range(KO // 4)]
            # first weight/x chunks split in half so the very first matmuls
            # wait on ~0.9 MB of DMA instead of 1.8 MB
            nc.sync.dma_start(w[0][:, 0:2, :], wqkv[:, 0:2, :])

            def wslice(k, m):
                return w[k // 4][:, k % 4, m * P:(m + 1) * P]

            pending_tr = [None]
            for sq in range(NSQ):
                cols = slice(sq * SQ, (sq + 1) * SQ)
                # allocate in eviction-pair order so the banks freed first by
                # the RoPE chain are the ones phase 2's first tiles land on
                pq = [None] * 6
                for m in (0, 3, 1, 4, 2, 5):
                    pq[m] = pp1.tile([P, SQ], F32, tag=f"p{m}", name=f"p{m}_{sq}")
                for xb in range(KO // XB):
                    xk = xpool.tile([P, XB, SQ], F16, tag="xt", name=f"x{sq}_{xb}")
                    if sq == 0 and xb == 0:
                        nc.sync.dma_start(xk[:, 0:2, :], xT[:, 0, 0:2, :])
                        nc.sync.dma_start(w[0][:, 2:4, :], wqkv[:, 2:4, :])
                        nc.sync.dma_start(xk[:, 2:4, :], xT[:, 0, 2:4, :])
                    else:
                        nc.sync.dma_start(xk[:],
                                          xT[:, sq, xb * XB:(xb + 1) * XB, :])
                    if sq == 0 and xb + 1 < KO // 4:
                        nc.sync.dma_start(w[xb + 1][:],
                                          wqkv[:, 4 * (xb + 1):4 * (xb + 2), :])
                    for kk in range(XB):
                        k = xb * XB + kk
                        for m in M_ORDER:
                            nc.tensor.matmul(pq[m][:], wslice(k, m), xk[:, kk, :],
                                             start=(k == 0), stop=(k == KO - 1))
                    if xb == 0 and pending_tr[0] is not None:
                        pending_tr[0]()
                        pending_tr[0] = None

                # RoPE eviction. m-tile pairs: (0,3)->(q0,q1), (1,4)->(q2,q3),
                # (2,5)->(k | v-halves). Copy-first: ALL six PSUM->SBUF
                # copies are emitted before any arithmetic (vector takes the
                # A tiles, scalar the B tiles, in parallel), so every PSUM
                # bank is free within ~2us of the last matmul and the next
                # sq tile / phase 2 never stalls behind the RoPE math.
                pas, pbs = [], []
                for i in range(3):
                    pa = rpool.tile([P, SQ], F32, tag=f"pa{i}", name=f"pa{i}")
                    pb = rpool.tile([P, SQ], F32, tag=f"pb{i}", name=f"pb{i}")
                    nc.vector.tensor_copy(pa[:], pq[i][:])
                    nc.scalar.copy(pb[:], pq[i + 3][:])
                    pas.append(pa)
                    pbs.append(pb)
                # v passthrough immediately after the evictions: the v
                # transposes (PE queue) and everything behind them would
                # otherwise wait for the whole serial RoPE arithmetic chain
                nc.vector.tensor_copy(vTsb[0:64, cols], pas[2][64:128])
                nc.scalar.copy(vTsb[64:128, cols], pbs[2][64:128])
                for i, (h0, h1) in enumerate(((0, 1), (2, 3), (4, 5))):
                    pa, pb = pas[i], pbs[i]
                    tac = rpool.tile([P, SQ], F32, tag="tac")   # A*cos
                    tas = rpool.tile([P, SQ], F32, tag="tas")   # A*sin
                    tbs = rpool.tile([P, SQ], F32, tag="tbs")   # B*sin
                    tbc = rpool.tile([P, SQ], F32, tag="tbc")   # B*cos
                    nc.vector.tensor_tensor(tac[:], pa[:], cos_sb[:, cols],
                                            MULT)
                    nc.vector.tensor_tensor(tas[:], pa[:], sin_sb[:, cols],
                                            MULT)
                    nc.vector.tensor_tensor(tbs[:], pb[:], sin_sb[:, cols],
                                            MULT)
                    nc.vector.tensor_tensor(tbc[:], pb[:], cos_sb[:, cols],
                                            MULT)
                    if i == 2:
                        dests = ((slice(0, 64), kTsb[0:64, cols],
                                  kTsb[64:128, cols]),)
                    else:
                        h0q, h1q = 2 * i, 2 * i + 1
                        dests = ((slice(0, 64), qsb[0:64, h0q, cols],
                                  qsb[64:128, h0q, cols]),
                                 (slice(64, 128), qsb[0:64, h1q, cols],
                                  qsb[64:128, h1q, cols]))
                    for half, dre, dim_ in dests:
                        nc.vector.tensor_tensor(dre, tac[half], tbs[half], SUB)
                        nc.vector.tensor_tensor(dim_, tas[half], tbc[half], ADD)

                # transpose this quarter's v chunks: vT [128, s] -> v [s, 128]
                # (deferred into the next sq tile's matmul stream so the PE
                # doesn't stall here waiting for the RoPE vector ops)
                def mk_transposes(sq=sq):
                    def emit():
                        for t in range(4 * sq, 4 * sq + 4):
                            ptr = pp1.tile([P, P], F32, tag="ptr", bufs=2,
                                           name=f"ptr{t}")
                            nc.tensor.transpose(ptr[:],
                                                vTsb[:, t * P:(t + 1) * P],
                                                idn[:])
                            nc.scalar.copy(vsb[:, t, :], ptr[:])
                    return emit
                pending_tr[0] = mk_transposes()
                if sq == NSQ - 1:
                    pending_tr[0]()
                    pending_tr[0] = None

        wopool = stack.enter_context(tc.tile_pool(name="wopool", bufs=1))
        wo2_sb = wopool.tile([P, KO, SQ], F16)
        nc.sync.dma_start(wo2_sb[:], wo2[:])

        # ------- Phase 2+3: causal GQA attention + column-sharded out proj ----
        # emit_attn(j) ends by exporting normalized ctx and triggering its
        # AllGather; emit_p3l(j) stages the gathered 4096-feature ctx and
        # multiplies it against the wo column-shard, writing final output
        # rows directly (no ReduceScatter, no bounce copies)
        with tc.tile_pool(name="ag", bufs=2) as agpool, \
             tc.tile_pool(name="os", bufs=4) as ospool, \
             tc.tile_pool(name="ps2", bufs=1, space="PSUM") as pp2:

            def emit_attn(j):
                nks = 4 * (j + 1)
                ctx_sb = cxpool.tile([P, NH_LOC, SQ], F16, tag="cx",
                                     name=f"cx{j}")
                for hp in range(2):
                    h0, h1 = 2 * hp, 2 * hp + 1
                    ctx0 = pp2.tile([P, SQ], F32, tag="ctx", bufs=2,
                                    name=f"ctx{j}_{h0}")
                    ctx1 = pp2.tile([P, SQ], F32, tag="ctx", bufs=2,
                                    name=f"ctx{j}_{h1}")
                    # softmax denominator accumulated on the Vector engine
                    # (fp16 operands run at the 2x DVE rate); saves two PE
                    # matmuls per key tile
                    acc = stpool.tile([P, 2 * SQ], F16, tag="acc", bufs=2,
                                      name=f"acc{j}_{hp}")

                    # software pipeline: scores/exp run 2 tiles ahead of PV.
                    # Diagonal-block tiles (r = t-4j > 0) only attend queries
                    # q >= 128r, so scores/exp/mask/acc/PV are restricted to
                    # that column range (the excluded columns are exactly the
                    # fully-masked ones; acc/ctx keep their per-element
                    # accumulation correct because t=0 is always full-width)
                    def do_scores(t, j=j, h0=h0, h1=h1, acc=acc):
                        r = t - 4 * j if t >= 4 * j else 0
                        q0 = 128 * r
                        qc = slice(j * SQ + q0, (j + 1) * SQ)
                        ps_s = pp2.tile([P, 2 * SQ], F32, tag="s", bufs=2,
                                        name=f"s{j}_{h0}_{t}")
                        kt = kTsb[:, t * P:(t + 1) * P]
                        nc.tensor.matmul(ps_s[:, q0:SQ], kt, qsb[:, h0, qc],
                                         start=True, stop=True)
                        nc.tensor.matmul(ps_s[:, SQ + q0:], kt,
                                         qsb[:, h1, qc],
                                         start=True, stop=True)
                        pT = ptpool.tile([P, 2 * SQ], F16, tag="pT",
                                         name=f"pT{j}_{h0}_{t}")
                        # bias -7 keeps exp within fp16 range (max observed
                        # score*scale is ~11.5); numerator and denominator
                        # scale by the same e^-7, so softmax is unchanged
                        if q0 == 0:
                            nc.scalar.activation(pT[:], ps_s[:], EXP,
                                                 scale=SCALE, bias=expbias[:])
                        else:
                            nc.scalar.activation(pT[:, q0:SQ], ps_s[:, q0:SQ],
                                                 EXP, scale=SCALE,
                                                 bias=expbias[:])
                            nc.scalar.activation(pT[:, SQ + q0:],
                                                 ps_s[:, SQ + q0:], EXP,
                                                 scale=SCALE, bias=expbias[:])
                        if t >= 4 * j:
                            if q0 == 0:
                                nc.vector.tensor_tensor(
                                    pT[:], pT[:], mask_sb[:, r, :], MULT)
                            else:
                                nc.vector.tensor_tensor(
                                    pT[:, q0:SQ], pT[:, q0:SQ],
                                    mask_sb[:, r, q0:SQ], MULT)
                                nc.vector.tensor_tensor(
                                    pT[:, SQ + q0:], pT[:, SQ + q0:],
                                    mask_sb[:, r, SQ + q0:], MULT)
                        if t == 0:
                            nc.vector.tensor_copy(acc[:], pT[:])
                        elif q0 == 0:
                            nc.vector.tensor_tensor(acc[:], acc[:], pT[:],
                                                    ADD)
                        else:
                            nc.vector.tensor_tensor(acc[:, q0:SQ],
                                                    acc[:, q0:SQ],
                                                    pT[:, q0:SQ], ADD)
                            nc.vector.tensor_tensor(acc[:, SQ + q0:],
                                                    acc[:, SQ + q0:],
                                                    pT[:, SQ + q0:], ADD)
                        return pT

                    def do_pv(t, pT, ctx0=ctx0, ctx1=ctx1, nks=nks, j=j):
                        r = t - 4 * j if t >= 4 * j else 0
                        q0 = 128 * r
                        vt = vsb[:, t, :]
                        nc.tensor.matmul(ctx0[:, q0:], vt, pT[:, q0:SQ],
                                         start=(t == 0), stop=(t == nks - 1))
                        nc.tensor.matmul(ctx1[:, q0:], vt, pT[:, SQ + q0:],
                                         start=(t == 0), stop=(t == nks - 1))

                    pend = {}
                    for t in range(nks):
                        pend[t] = do_scores(t)
                        if t >= 2:
                            do_pv(t - 2, pend.pop(t - 2))
                    for t in (nks - 2, nks - 1):
                        do_pv(t, pend.pop(t))

                    # broadcast the denominator across partitions with an
                    # all-ones stationary, then normalize
                    bc = pp2.tile([P, 2 * SQ], F32, tag="bc", bufs=1,
                                  name=f"bc{j}_{hp}")
                    nc.tensor.matmul(bc[:, 0:SQ], ones128[:], acc[:, 0:SQ],
                                     start=True, stop=True)
                    nc.tensor.matmul(bc[:, SQ:], ones128[:], acc[:, SQ:],
                                     start=True, stop=True)
                    rc = stpool.tile([P, 2 * SQ], F32, tag="rc",
                                     name=f"rc{j}_{hp}")
                    nc.vector.reciprocal_approx_fast(rc[:], bc[:])
                    nc.vector.tensor_tensor(ctx_sb[:, h0, :], ctx0[:],
                                            rc[:, 0:SQ], MULT)
                    nc.vector.tensor_tensor(ctx_sb[:, h1, :], ctx1[:],
                                            rc[:, SQ:], MULT)
                    # export this head-pair's normalized ctx immediately so
                    # the AllGather can trigger right at attention end
                    nc.sync.dma_start(
                        agins[j][:].rearrange("(h p) s -> p h s",
                                              h=NH_LOC)[:, h0:h1 + 1, :],
                        ctx_sb[:, h0:h1 + 1, :])

                # gather all cores' ctx features for this seq tile
                # (0.5 MB in, 4 MB out per core)
                nc.gpsimd.collective_compute(
                    "AllGather", mybir.AluOpType.bypass,
                    replica_groups=[list(range(NCORES))],
                    ins=[agins[j][:]], outs=[agouts[j][:]])

            def emit_p3l(j):
                # local out projection for this seq tile: out[s, o_shard] =
                # sum_f ctx_all[f, s] * wo2[f, o]; fp32 PSUM over all 4096
                # features (numerically cleaner than the old fp16 partial
                # sum via the collective's CCE adds)
                agsb = agpool.tile([P, KO, SQ], F16, tag="agsb",
                                   name=f"agsb{j}")
                src = agouts[j][:].rearrange("(f p) s -> p f s", p=P)
                # staged in quarters so the first s-tile sweep pipelines
                # with the tail of the transfer
                for b in range(4):
                    nc.sync.dma_start(agsb[:, 8 * b:8 * (b + 1), :],
                                      src[:, 8 * b:8 * (b + 1), :])
                for st in range(4):
                    # po2 shares the (double-buffered) s-tag banks: the next
                    # sweep's matmuls overlap this one's eviction
                    po2 = pp2.tile([P, 2 * SQ], F32, tag="s", bufs=2,
                                   name=f"po2_{j}_{st}")
                    for fc in range(KO):
                        nc.tensor.matmul(
                            po2[:, 0:SQ],
                            agsb[:, fc, st * P:(st + 1) * P],
                            wo2_sb[:, fc, :],
                            start=(fc == 0), stop=(fc == KO - 1))
                    osb = ospool.tile([P, SQ], F16, tag="osb",
                                      name=f"osb{j}_{st}")
                    nc.vector.tensor_copy(osb[:], po2[:, 0:SQ])
                    nc.sync.dma_start(outs[j][st * P:(st + 1) * P, :],
                                      osb[:])

            # interleave so each tile's AllGather + staging hides under the
            # next tile's attention; the final p3l pair runs back-to-back
            emit_attn(JORDER[0])
            emit_attn(JORDER[1])
            emit_p3l(JORDER[0])
            emit_attn(JORDER[2])
            emit_p3l(JORDER[1])
            emit_attn(JORDER[3])
            emit_p3l(JORDER[2])
            emit_p3l(JORDER[3])

    nc.compile()
    _CACHE["nc"] = nc
    return nc


def _prep_inputs(x, wq, wk, wv, wo, freqs_cos, freqs_sin):
    """Host-side sharding + layout prep. Returns in_maps for the 8 cores."""
    x = np.asarray(x, dtype=np.float32)
    wq = np.asarray(wq, dtype=np.float32)
    wk = np.asarray(wk, dtype=np.float32)
    wv = np.asarray(wv, dtype=np.float32)
    wo = np.asarray(wo, dtype=np.float32)
    freqs_cos = np.asarray(freqs_cos, dtype=np.float32)
    freqs_sin = np.asarray(freqs_sin, dtype=np.float32)

    # xT in [P, NSQ, KO, SQ] layout: element (d, s), d = ko*128 + p, s = sq*SQ + s'
    xT = np.ascontiguousarray(
        x[0].T.reshape(KO, P, NSQ, SQ).transpose(1, 2, 0, 3)).astype(np.float16)

    # rotate-half permutation within a head: [0,2,4,...126, 1,3,...,127]
    perm = np.concatenate([np.arange(0, HEAD_DIM, 2), np.arange(1, HEAD_DIM, 2)])

    # cos/sin tables transposed and duplicated across both 64-row halves
    cosT = np.ascontiguousarray(freqs_cos.T)  # [64, SEQ]
    sinT = np.ascontiguousarray(freqs_sin.T)
    cos2 = np.concatenate([cosT, cosT], axis=0)  # [128, SEQ]
    sin2 = np.concatenate([sinT, sinT], axis=0)

    # causal mask tiles: mask_r[i, jl] = 1 if jl - i >= 128*r, duplicated
    # across both halves of the head-pair score tile
    i_idx = np.arange(P)[:, None]
    j_idx = np.arange(SQ)[None, :]
    masks = np.stack([(j_idx - i_idx >= P * r).astype(np.float32)
                      for r in range(4)], axis=0)  # [4, 128, SQ]
    masks_l = np.ascontiguousarray(
        np.concatenate([masks, masks], axis=2).transpose(1, 0, 2)
    ).astype(np.float16)  # [P,4,2SQ]

    in_maps = []
    for c in range(NCORES):
        # fused qkv weight rows, permuted for RoPE (re/im separated by m-tile)
        qh = [wq[(4 * c + h) * HEAD_DIM:(4 * c + h + 1) * HEAD_DIM][perm]
              for h in range(NH_LOC)]  # each [128, DIM], rows [re(64); im(64)]
        kh = wk[c * HEAD_DIM:(c + 1) * HEAD_DIM][perm]  # [128, DIM]
        vh = wv[c * HEAD_DIM:(c + 1) * HEAD_DIM]        # [128, DIM] original order
        rows = np.empty((MQKV, DIM), dtype=np.float32)
        rows[0:64] = qh[0][0:64]        # tile0: q0 re | q1 re
        rows[64:128] = qh[1][0:64]
        rows[128:192] = qh[2][0:64]     # tile1: q2 re | q3 re
        rows[192:256] = qh[3][0:64]
        rows[256:320] = kh[0:64]        # tile2: k re | v dims 0:64
        rows[320:384] = vh[0:64]
        rows[384:448] = qh[0][64:128]   # tile3: q0 im | q1 im
        rows[448:512] = qh[1][64:128]
        rows[512:576] = qh[2][64:128]   # tile4: q2 im | q3 im
        rows[576:640] = qh[3][64:128]
        rows[640:704] = kh[64:128]      # tile5: k im | v dims 64:128
        rows[704:768] = vh[64:128]
        wqkvT = np.ascontiguousarray(
            rows.T.reshape(KO, P, MQKV).transpose(1, 0, 2)
        ).astype(np.float16)  # [P, KO, MQKV]

        # wo COLUMN shard, feature-major: wo2[p, k, o] = wo[512c+o, 128k+p]
        wo2T = np.ascontiguousarray(
            wo[c * SQ:(c + 1) * SQ, :].T
            .reshape(KO, P, SQ).transpose(1, 0, 2)).astype(np.float16)

        in_maps.append({
            "xt": xT,
            "wqkv": wqkvT,
            "wo2": wo2T,
            "cost": cos2,
            "sint": sin2,
            "masks": masks_l,
        })
    return in_maps


def run(inputs, trace=False, tmpdir=None):
    """Compile (cached), run on 8 cores, return (output, BassKernelResults)."""
    from concourse.bass_utils import run_bass_kernel_spmd

    nc = _build()
    in_maps = _prep_inputs(**inputs)
    res = run_bass_kernel_spmd(nc, in_maps, list(range(NCORES)),
                               trace=trace, tmpdir=tmpdir)
    out = np.empty((BATCH, SEQ, DIM), dtype=np.float32)
    for c in range(NCORES):
        for j in range(NSQ):
            out[0, j * SQ:(j + 1) * SQ, c * SQ:(c + 1) * SQ] = np.asarray(
                res.results[c][f"o{j}"], dtype=np.float32)
    return out, res


def kernel(**inputs) -> np.ndarray:
    out, _ = run(inputs)
    return out



# revision 66
# speedup vs baseline: 1.1897x; 1.0126x over previous
"""Tensor-parallel GQA attention kernel for 8 Trainium2 NeuronCores.

Sharding: head-parallel. Core c computes q heads [4c, 4c+4) and kv head c
(GQA group). The output projection is COLUMN-sharded: after each seq tile's
attention, a small AllGather (0.5 MB/core) exchanges the normalized ctx
features, and every core contracts all 4096 features against its wo
column-shard, writing final output columns [512c, 512c+512) directly. The
host reassembles the 4 x 8 (seq x column) blocks. This replaces the old
row-sharded partial-sum ReduceScatter chain (16 MB/core of collective
traffic that serialized ~90us past the last matmul).

Attention processes query heads in pairs; the QKV projection, scores, PV,
and denominator matmuls all use fp16 operands (LDWEIGHTS for 16-bit
stationaries is ~2x faster, giving full 213ns/matmul cadence), while PSUM
accumulation stays fp32 and the output projection keeps wo in f32r. The
softmax denominator is accumulated exactly on the PE via an all-ones fp16
stationary into a broadcast PSUM tile.
"""

import math
import sys

import numpy as np

sys.path.insert(0, "/opt/trn_rl_repo")

# ---- problem constants (hardcoded per harness contract) ----
DIM = 4096
N_HEADS = 32
N_KV_HEADS = 8
HEAD_DIM = 128
N_REP = 4
SEQ = 2048
BATCH = 1
NCORES = 8

P = 128
KO = DIM // P        # 32 contraction chunks
SQ = 512             # seq tile width (matmul moving free dim)
NSQ = SEQ // SQ      # 4
NKS = SEQ // P       # 16 key tiles of 128
NH_LOC = N_HEADS // NCORES   # 4 local q heads
MQKV = NH_LOC * HEAD_DIM + 2 * HEAD_DIM  # 768 rows of fused qkv projection
SCALE = 1.0 / math.sqrt(HEAD_DIM)
OSH = SQ // NCORES   # 64 seq rows per core from each ReduceScatter

XB = 4               # k-chunks per xT load (1 MB DMAs)
JORDER = (1, 2, 3, 0)  # j=0 last: its partials are the cheapest to compute
                       # at the tail, and processing j=1 first avoids a
                       # framework serialization of the first RS against the
                       # following tile's partial writes seen with j=0 first

_CACHE = {}


def _build():
    """Build and compile the Bass kernel once per process."""
    if "nc" in _CACHE:
        return _CACHE["nc"]

    import concourse.bacc as bacc
    import concourse.mybir as mybir
    import concourse.tile as tile
    from concourse.masks import make_identity
    from contextlib import ExitStack

    F32 = mybir.dt.float32
    F32R = mybir.dt.float32r
    F16 = mybir.dt.float16
    MULT = mybir.AluOpType.mult
    ADD = mybir.AluOpType.add
    SUB = mybir.AluOpType.subtract
    EXP = mybir.ActivationFunctionType.Exp

    nc = bacc.Bacc(None, target_bir_lowering=False, debug=False)

    xT = nc.declare_dram_parameter("xt", [P, NSQ, KO, SQ], F16, isOutput=False)
    wqkv = nc.declare_dram_parameter("wqkv", [P, KO, MQKV], F16, isOutput=False)
    # wo column-shard, feature-major: wo2[p, k, o] = wo[512c+o, 128k+p]
    wo2 = nc.declare_dram_parameter("wo2", [P, KO, SQ], F16, isOutput=False)
    cosd = nc.declare_dram_parameter("cost", [P, SEQ], F32, isOutput=False)
    sind = nc.declare_dram_parameter("sint", [P, SEQ], F32, isOutput=False)
    maskd = nc.declare_dram_parameter("masks", [P, 4, 2 * SQ], F16, isOutput=False)
    # each core emits the full seq rows of its 512 output COLUMNS: the out
    # projection is done locally against a wo column-shard after a small
    # AllGather of ctx (0.5 MB/core/tile), replacing the 16 MB/core serial
    # ReduceScatter chain that used to trail the compute by ~90us
    outs = [nc.declare_dram_parameter(f"o{j}", [SQ, SQ], F16, isOutput=True)
            for j in range(NSQ)]

    with tile.TileContext(nc) as tc, ExitStack() as stack:
        singles = stack.enter_context(tc.tile_pool(name="singles", bufs=1))
        dram = stack.enter_context(tc.tile_pool(name="dram", bufs=1, space="DRAM"))

        # AllGather staging: agin = my 512 ctx features (feature-major) for
        # this seq tile; agout = all 4096 features, rank-concatenated
        agins = [dram.tile([NH_LOC * P, SQ], F16, name=f"agin{j}")
                 for j in range(NSQ)]
        agouts = [dram.tile([DIM, SQ], F16, name=f"agout{j}",
                            addr_space="Shared") for j in range(NSQ)]
        # warm-up collective, SAME shape as the real per-tile AllGathers: the
        # first collective after load pays an ~11us ncfw cold-start and the
        # first AG of a given size measured ~10us slower than later ones;
        # absorb both during phase 1 (result unused, input garbage is fine)
        warmin = dram.tile([NH_LOC * P, SQ], F16, name="warmin")
        warmout = dram.tile([DIM, SQ], F16, name="warmout",
                            addr_space="Shared")

        idn = singles.tile([P, P], F32)
        make_identity(nc, idn)

        warmsb = singles.tile([8, P], F16)
        nc.vector.memset(warmsb[:], 0.0)
        nc.sync.dma_start(warmin[0:8, 0:P], warmsb[:])
        nc.gpsimd.collective_compute(
            "AllGather", mybir.AluOpType.bypass,
            replica_groups=[list(range(NCORES))],
            ins=[warmin[:]], outs=[warmout[:]])

        ones_f = singles.tile([P, P], F32)
        nc.vector.memset(ones_f[:], 1.0)
        expbias = singles.tile([P, 1], F32)
        nc.vector.memset(expbias[:], -7.0)
        ones128 = singles.tile([P, P], F16)
        nc.vector.tensor_copy(ones128[:], ones_f[:])

        # attention operands, resident across phases 1-2
        qsb = singles.tile([P, NH_LOC, SEQ], F16)   # per head: rows 0:64 re, 64:128 im
        kTsb = singles.tile([P, SEQ], F16)
        vsb = singles.tile([P, NKS, HEAD_DIM], F16)
        # mask lives through phase 1 so its DMA can be issued early: the
        # first attention tile (j=0, t=0) is diagonal and needs it at once
        mask_sb = singles.tile([P, 4, 2 * SQ], F16)
        # mask loads on the gpsimd queue behind the warmup AllGather: off
        # the bandwidth-critical sync stream, done ~100us in, needed ~230us
        nc.gpsimd.dma_start(mask_sb[:], maskd[:])
        # the phase-2 softmax-path pools are created BEFORE phase 1 so their
        # tiles get dedicated addresses: created afterwards, they reuse the
        # freed phase-1 scratch and the first exp inherits a ~10us WAR wait
        # on the last sq tile's serial RoPE arithmetic chain
        ptpool = stack.enter_context(tc.tile_pool(name="pt", bufs=3))
        stpool = stack.enter_context(tc.tile_pool(name="st", bufs=2))
        cxpool = stack.enter_context(tc.tile_pool(name="cx", bufs=2))

        # ---------------- Phase 1: fused QKV projection + RoPE ----------------
        # m-tile order chosen so PSUM tiles are revisited in the order the
        # RoPE eviction frees them (pairs (0,3), (1,4), (2,5)).
        M_ORDER = (0, 3, 1, 4, 2, 5)
        with tc.tile_pool(name="wq", bufs=1) as wpool, \
             tc.tile_pool(name="xtp", bufs=2) as xpool, \
             tc.tile_pool(name="rt", bufs=2) as rpool, \
             tc.tile_pool(name="ps1", bufs=1, space="PSUM") as pp1:
            cos_sb = wpool.tile([P, SEQ], F32, tag="cos", name="cos_sb")
            sin_sb = wpool.tile([P, SEQ], F32, tag="sin", name="sin_sb")
            vTsb = wpool.tile([P, SEQ], F32, tag="vT", name="vTsb")

            # weight tiles allocated up front; DMAs interleaved with the x
            # stream of the first sq tile so the first matmul starts after
            # ~2.6 MB instead of 13 MB
            w = [wpool.tile([P, 4, MQKV], F16, tag=f"w{g}", name=f"w{g}")
                 for g in range(KO // 4)]
            # first weight/x chunks split in half so the very first matmuls
            # wait on ~0.9 MB of DMA instead of 1.8 MB
            nc.sync.dma_start(w[0][:, 0:2, :], wqkv[:, 0:2, :])

            def wslice(k, m):
                return w[k // 4][:, k % 4, m * P:(m + 1) * P]

            pending_tr = [None]
            for sq in range(NSQ):
                cols = slice(sq * SQ, (sq + 1) * SQ)
                # allocate in eviction-pair order so the banks freed first by
                # the RoPE chain are the ones phase 2's first tiles land on
                pq = [None] * 6
                for m in (0, 3, 1, 4, 2, 5):
                    pq[m] = pp1.tile([P, SQ], F32, tag=f"p{m}", name=f"p{m}_{sq}")
                for xb in range(KO // XB):
                    xk = xpool.tile([P, XB, SQ], F16, tag="xt", name=f"x{sq}_{xb}")
                    if sq == 0 and xb == 0:
                        nc.sync.dma_start(xk[:, 0:2, :], xT[:, 0, 0:2, :])
                        nc.sync.dma_start(w[0][:, 2:4, :], wqkv[:, 2:4, :])
                        nc.sync.dma_start(xk[:, 2:4, :], xT[:, 0, 2:4, :])
                    else:
                        nc.sync.dma_start(xk[:],
                                          xT[:, sq, xb * XB:(xb + 1) * XB, :])
                    if sq == 0 and xb + 1 < KO // 4:
                        nc.sync.dma_start(w[xb + 1][:],
                                          wqkv[:, 4 * (xb + 1):4 * (xb + 2), :])
                    if sq == 0 and xb == 1:
                        # cos/sin aren't needed until the first RoPE eviction
                        # (~45us in): issue them behind the first two x/w
                        # chunk groups
                        nc.sync.dma_start(cos_sb[:], cosd[:])
                        nc.sync.dma_start(sin_sb[:], sind[:])
                    for kk in range(XB):
                        k = xb * XB + kk
                        for m in M_ORDER:
                            nc.tensor.matmul(pq[m][:], wslice(k, m), xk[:, kk, :],
                                             start=(k == 0), stop=(k == KO - 1))
                    if xb == 0 and pending_tr[0] is not None:
                        pending_tr[0]()
                        pending_tr[0] = None

                # RoPE eviction. m-tile pairs: (0,3)->(q0,q1), (1,4)->(q2,q3),
                # (2,5)->(k | v-halves). Copy-first: ALL six PSUM->SBUF
                # copies are emitted before any arithmetic (vector takes the
                # A tiles, scalar the B tiles, in parallel), so every PSUM
                # bank is free within ~2us of the last matmul and the next
                # sq tile / phase 2 never stalls behind the RoPE math.
                pas, pbs = [], []
                for i in range(3):
                    pa = rpool.tile([P, SQ], F32, tag=f"pa{i}", name=f"pa{i}")
                    pb = rpool.tile([P, SQ], F32, tag=f"pb{i}", name=f"pb{i}")
                    nc.vector.tensor_copy(pa[:], pq[i][:])
                    nc.scalar.copy(pb[:], pq[i + 3][:])
                    pas.append(pa)
                    pbs.append(pb)
                # v passthrough immediately after the evictions: the v
                # transposes (PE queue) and everything behind them would
                # otherwise wait for the whole serial RoPE arithmetic chain
                nc.vector.tensor_copy(vTsb[0:64, cols], pas[2][64:128])
                nc.scalar.copy(vTsb[64:128, cols], pbs[2][64:128])
                for i, (h0, h1) in enumerate(((0, 1), (2, 3), (4, 5))):
                    pa, pb = pas[i], pbs[i]
                    tac = rpool.tile([P, SQ], F32, tag="tac")   # A*cos
                    tas = rpool.tile([P, SQ], F32, tag="tas")   # A*sin
                    tbs = rpool.tile([P, SQ], F32, tag="tbs")   # B*sin
                    tbc = rpool.tile([P, SQ], F32, tag="tbc")   # B*cos
                    nc.vector.tensor_tensor(tac[:], pa[:], cos_sb[:, cols],
                                            MULT)
                    nc.vector.tensor_tensor(tas[:], pa[:], sin_sb[:, cols],
                                            MULT)
                    nc.vector.tensor_tensor(tbs[:], pb[:], sin_sb[:, cols],
                                            MULT)
                    nc.vector.tensor_tensor(tbc[:], pb[:], cos_sb[:, cols],
                                            MULT)
                    if i == 2:
                        dests = ((slice(0, 64), kTsb[0:64, cols],
                                  kTsb[64:128, cols]),)
                    else:
                        h0q, h1q = 2 * i, 2 * i + 1
                        dests = ((slice(0, 64), qsb[0:64, h0q, cols],
                                  qsb[64:128, h0q, cols]),
                                 (slice(64, 128), qsb[0:64, h1q, cols],
                                  qsb[64:128, h1q, cols]))
                    for half, dre, dim_ in dests:
                        nc.vector.tensor_tensor(dre, tac[half], tbs[half], SUB)
                        nc.vector.tensor_tensor(dim_, tas[half], tbc[half], ADD)

                # transpose this quarter's v chunks: vT [128, s] -> v [s, 128]
                # (deferred into the next sq tile's matmul stream so the PE
                # doesn't stall here waiting for the RoPE vector ops)
                def mk_transposes(sq=sq):
                    def emit():
                        for t in range(4 * sq, 4 * sq + 4):
                            ptr = pp1.tile([P, P], F32, tag="ptr", bufs=2,
                                           name=f"ptr{t}")
                            nc.tensor.transpose(ptr[:],
                                                vTsb[:, t * P:(t + 1) * P],
                                                idn[:])
                            nc.scalar.copy(vsb[:, t, :], ptr[:])
                    return emit
                pending_tr[0] = mk_transposes()
                if sq == NSQ - 1:
                    pending_tr[0]()
                    pending_tr[0] = None

        wopool = stack.enter_context(tc.tile_pool(name="wopool", bufs=1))
        wo2_sb = wopool.tile([P, KO, SQ], F16)
        nc.sync.dma_start(wo2_sb[:], wo2[:])

        # ------- Phase 2+3: causal GQA attention + column-sharded out proj ----
        # emit_attn(j) ends by exporting normalized ctx and triggering its
        # AllGather; emit_p3l(j) stages the gathered 4096-feature ctx and
        # multiplies it against the wo column-shard, writing final output
        # rows directly (no ReduceScatter, no bounce copies)
        with tc.tile_pool(name="ag", bufs=2) as agpool, \
             tc.tile_pool(name="os", bufs=4) as ospool, \
             tc.tile_pool(name="ps2", bufs=1, space="PSUM") as pp2:

            def emit_attn(j):
                nks = 4 * (j + 1)
                ctx_sb = cxpool.tile([P, NH_LOC, SQ], F16, tag="cx",
                                     name=f"cx{j}")
                for hp in range(2):
                    h0, h1 = 2 * hp, 2 * hp + 1
                    ctx0 = pp2.tile([P, SQ], F32, tag="ctx", bufs=2,
                                    name=f"ctx{j}_{h0}")
                    ctx1 = pp2.tile([P, SQ], F32, tag="ctx", bufs=2,
                                    name=f"ctx{j}_{h1}")
                    # softmax denominator accumulated on the Vector engine
                    # (fp16 operands run at the 2x DVE rate); saves two PE
                    # matmuls per key tile
                    acc = stpool.tile([P, 2 * SQ], F16, tag="acc", bufs=2,
                                      name=f"acc{j}_{hp}")

                    # software pipeline: scores/exp run 2 tiles ahead of PV.
                    # Diagonal-block tiles (r = t-4j > 0) only attend queries
                    # q >= 128r, so scores/exp/mask/acc/PV are restricted to
                    # that column range (the excluded columns are exactly the
                    # fully-masked ones; acc/ctx keep their per-element
                    # accumulation correct because t=0 is always full-width)
                    def do_scores(t, j=j, h0=h0, h1=h1, acc=acc):
                        r = t - 4 * j if t >= 4 * j else 0
                        q0 = 128 * r
                        qc = slice(j * SQ + q0, (j + 1) * SQ)
                        ps_s = pp2.tile([P, 2 * SQ], F32, tag="s", bufs=2,
                                        name=f"s{j}_{h0}_{t}")
                        kt = kTsb[:, t * P:(t + 1) * P]
                        nc.tensor.matmul(ps_s[:, q0:SQ], kt, qsb[:, h0, qc],
                                         start=True, stop=True)
                        nc.tensor.matmul(ps_s[:, SQ + q0:], kt,
                                         qsb[:, h1, qc],
                                         start=True, stop=True)
                        pT = ptpool.tile([P, 2 * SQ], F16, tag="pT",
                                         name=f"pT{j}_{h0}_{t}")
                        # bias -7 keeps exp within fp16 range (max observed
                        # score*scale is ~11.5); numerator and denominator
                        # scale by the same e^-7, so softmax is unchanged
                        if q0 == 0:
                            nc.scalar.activation(pT[:], ps_s[:], EXP,
                                                 scale=SCALE, bias=expbias[:])
                        else:
                            nc.scalar.activation(pT[:, q0:SQ], ps_s[:, q0:SQ],
                                                 EXP, scale=SCALE,
                                                 bias=expbias[:])
                            nc.scalar.activation(pT[:, SQ + q0:],
                                                 ps_s[:, SQ + q0:], EXP,
                                                 scale=SCALE, bias=expbias[:])
                        if t >= 4 * j:
                            if q0 == 0:
                                nc.vector.tensor_tensor(
                                    pT[:], pT[:], mask_sb[:, r, :], MULT)
                            else:
                                nc.vector.tensor_tensor(
                                    pT[:, q0:SQ], pT[:, q0:SQ],
                                    mask_sb[:, r, q0:SQ], MULT)
                                nc.vector.tensor_tensor(
                                    pT[:, SQ + q0:], pT[:, SQ + q0:],
                                    mask_sb[:, r, SQ + q0:], MULT)
                        if t == 0:
                            nc.vector.tensor_copy(acc[:], pT[:])
                        elif q0 == 0:
                            nc.vector.tensor_tensor(acc[:], acc[:], pT[:],
                                                    ADD)
                        else:
                            nc.vector.tensor_tensor(acc[:, q0:SQ],
                                                    acc[:, q0:SQ],
                                                    pT[:, q0:SQ], ADD)
                            nc.vector.tensor_tensor(acc[:, SQ + q0:],
                                                    acc[:, SQ + q0:],
                                                    pT[:, SQ + q0:], ADD)
                        return pT

                    def do_pv(t, pT, ctx0=ctx0, ctx1=ctx1, nks=nks, j=j):
                        r = t - 4 * j if t >= 4 * j else 0
                        q0 = 128 * r
                        vt = vsb[:, t, :]
                        nc.tensor.matmul(ctx0[:, q0:], vt, pT[:, q0:SQ],
                                         start=(t == 0), stop=(t == nks - 1))
                        nc.tensor.matmul(ctx1[:, q0:], vt, pT[:, SQ + q0:],
                                         start=(t == 0), stop=(t == nks - 1))

                    pend = {}
                    for t in range(nks):
                        pend[t] = do_scores(t)
                        if t >= 2:
                            do_pv(t - 2, pend.pop(t - 2))
                    for t in (nks - 2, nks - 1):
                        do_pv(t, pend.pop(t))

                    # broadcast the denominator across partitions with an
                    # all-ones stationary, then normalize
                    bc = pp2.tile([P, 2 * SQ], F32, tag="bc", bufs=1,
                                  name=f"bc{j}_{hp}")
                    nc.tensor.matmul(bc[:, 0:SQ], ones128[:], acc[:, 0:SQ],
                                     start=True, stop=True)
                    nc.tensor.matmul(bc[:, SQ:], ones128[:], acc[:, SQ:],
                                     start=True, stop=True)
                    rc = stpool.tile([P, 2 * SQ], F32, tag="rc",
                                     name=f"rc{j}_{hp}")
                    nc.vector.reciprocal_approx_fast(rc[:], bc[:])
                    nc.vector.tensor_tensor(ctx_sb[:, h0, :], ctx0[:],
                                            rc[:, 0:SQ], MULT)
                    nc.vector.tensor_tensor(ctx_sb[:, h1, :], ctx1[:],
                                            rc[:, SQ:], MULT)
                    # export this head-pair's normalized ctx immediately so
                    # the AllGather can trigger right at attention end
                    nc.sync.dma_start(
                        agins[j][:].rearrange("(h p) s -> p h s",
                                              h=NH_LOC)[:, h0:h1 + 1, :],
                        ctx_sb[:, h0:h1 + 1, :])

                # gather all cores' ctx features for this seq tile
                # (0.5 MB in, 4 MB out per core)
                nc.gpsimd.collective_compute(
                    "AllGather", mybir.AluOpType.bypass,
                    replica_groups=[list(range(NCORES))],
                    ins=[agins[j][:]], outs=[agouts[j][:]])

            def emit_p3l(j):
                # local out projection for this seq tile: out[s, o_shard] =
                # sum_f ctx_all[f, s] * wo2[f, o]; fp32 PSUM over all 4096
                # features (numerically cleaner than the old fp16 partial
                # sum via the collective's CCE adds)
                agsb = agpool.tile([P, KO, SQ], F16, tag="agsb",
                                   name=f"agsb{j}")
                src = agouts[j][:].rearrange("(f p) s -> p f s", p=P)
                # staged in quarters so the first s-tile sweep pipelines
                # with the tail of the transfer
                for b in range(4):
                    nc.sync.dma_start(agsb[:, 8 * b:8 * (b + 1), :],
                                      src[:, 8 * b:8 * (b + 1), :])
                for st in range(4):
                    # po2 shares the (double-buffered) s-tag banks: the next
                    # sweep's matmuls overlap this one's eviction
                    po2 = pp2.tile([P, 2 * SQ], F32, tag="s", bufs=2,
                                   name=f"po2_{j}_{st}")
                    for fc in range(KO):
                        nc.tensor.matmul(
                            po2[:, 0:SQ],
                            agsb[:, fc, st * P:(st + 1) * P],
                            wo2_sb[:, fc, :],
                            start=(fc == 0), stop=(fc == KO - 1))
                    osb = ospool.tile([P, SQ], F16, tag="osb",
                                      name=f"osb{j}_{st}")
                    nc.vector.tensor_copy(osb[:], po2[:, 0:SQ])
                    nc.sync.dma_start(outs[j][st * P:(st + 1) * P, :],
                                      osb[:])

            # interleave so each tile's AllGather + staging hides under the
            # next tile's attention; the final p3l pair runs back-to-back
            emit_attn(JORDER[0])
            emit_attn(JORDER[1])
            emit_p3l(JORDER[0])
            emit_attn(JORDER[2])
            emit_p3l(JORDER[1])
            emit_attn(JORDER[3])
            emit_p3l(JORDER[2])
            emit_p3l(JORDER[3])

    nc.compile()
    _CACHE["nc"] = nc
    return nc


def _prep_inputs(x, wq, wk, wv, wo, freqs_cos, freqs_sin):
    """Host-side sharding + layout prep. Returns in_maps for the 8 cores."""
    x = np.asarray(x, dtype=np.float32)
    wq = np.asarray(wq, dtype=np.float32)
    wk = np.asarray(wk, dtype=np.float32)
    wv = np.asarray(wv, dtype=np.float32)
    wo = np.asarray(wo, dtype=np.float32)
    freqs_cos = np.asarray(freqs_cos, dtype=np.float32)
    freqs_sin = np.asarray(freqs_sin, dtype=np.float32)

    # xT in [P, NSQ, KO, SQ] layout: element (d, s), d = ko*128 + p, s = sq*SQ + s'
    xT = np.ascontiguousarray(
        x[0].T.reshape(KO, P, NSQ, SQ).transpose(1, 2, 0, 3)).astype(np.float16)

    # rotate-half permutation within a head: [0,2,4,...126, 1,3,...,127]
    perm = np.concatenate([np.arange(0, HEAD_DIM, 2), np.arange(1, HEAD_DIM, 2)])

    # cos/sin tables transposed and duplicated across both 64-row halves
    cosT = np.ascontiguousarray(freqs_cos.T)  # [64, SEQ]
    sinT = np.ascontiguousarray(freqs_sin.T)
    cos2 = np.concatenate([cosT, cosT], axis=0)  # [128, SEQ]
    sin2 = np.concatenate([sinT, sinT], axis=0)

    # causal mask tiles: mask_r[i, jl] = 1 if jl - i >= 128*r, duplicated
    # across both halves of the head-pair score tile
    i_idx = np.arange(P)[:, None]
    j_idx = np.arange(SQ)[None, :]
    masks = np.stack([(j_idx - i_idx >= P * r).astype(np.float32)
                      for r in range(4)], axis=0)  # [4, 128, SQ]
    masks_l = np.ascontiguousarray(
        np.concatenate([masks, masks], axis=2).transpose(1, 0, 2)
    ).astype(np.float16)  # [P,4,2SQ]

    in_maps = []
    for c in range(NCORES):
        # fused qkv weight rows, permuted for RoPE (re/im separated by m-tile)
        qh = [wq[(4 * c + h) * HEAD_DIM:(4 * c + h + 1) * HEAD_DIM][perm]
              for h in range(NH_LOC)]  # each [128, DIM], rows [re(64); im(64)]
        kh = wk[c * HEAD_DIM:(c + 1) * HEAD_DIM][perm]  # [128, DIM]
        vh = wv[c * HEAD_DIM:(c + 1) * HEAD_DIM]        # [128, DIM] original order
        rows = np.empty((MQKV, DIM), dtype=np.float32)
        rows[0:64] = qh[0][0:64]        # tile0: q0 re | q1 re
        rows[64:128] = qh[1][0:64]
        rows[128:192] = qh[2][0:64]     # tile1: q2 re | q3 re
        rows[192:256] = qh[3][0:64]
        rows[256:320] = kh[0:64]        # tile2: k re | v dims 0:64
        rows[320:384] = vh[0:64]
        rows[384:448] = qh[0][64:128]   # tile3: q0 im | q1 im
        rows[448:512] = qh[1][64:128]
        rows[512:576] = qh[2][64:128]   # tile4: q2 im | q3 im
        rows[576:640] = qh[3][64:128]
        rows[640:704] = kh[64:128]      # tile5: k im | v dims 64:128
        rows[704:768] = vh[64:128]
        wqkvT = np.ascontiguousarray(
            rows.T.reshape(KO, P, MQKV).transpose(1, 0, 2)
        ).astype(np.float16)  # [P, KO, MQKV]

        # wo COLUMN shard, feature-major: wo2[p, k, o] = wo[512c+o, 128k+p]
        wo2T = np.ascontiguousarray(
            wo[c * SQ:(c + 1) * SQ, :].T
            .reshape(KO, P, SQ).transpose(1, 0, 2)).astype(np.float16)

        in_maps.append({
            "xt": xT,
            "wqkv": wqkvT,
            "wo2": wo2T,
            "cost": cos2,
            "sint": sin2,
            "masks": masks_l,
        })
    return in_maps


def run(inputs, trace=False, tmpdir=None):
    """Compile (cached), run on 8 cores, return (output, BassKernelResults)."""
    from concourse.bass_utils import run_bass_kernel_spmd

    nc = _build()
    in_maps = _prep_inputs(**inputs)
    res = run_bass_kernel_spmd(nc, in_maps, list(range(NCORES)),
                               trace=trace, tmpdir=tmpdir)
    out = np.empty((BATCH, SEQ, DIM), dtype=np.float32)
    for c in range(NCORES):
        for j in range(NSQ):
            out[0, j * SQ:(j + 1) * SQ, c * SQ:(c + 1) * SQ] = np.asarray(
                res.results[c][f"o{j}"], dtype=np.float32)
    return out, res


def kernel(**inputs) -> np.ndarray:
    out, _ = run(inputs)
    return out

